# revision 18
# baseline (speedup 1.0000x reference)
"""Bass/Trainium2 kernel for nn_MultiHeadAttention (B=4, S=2048, E=512, H=8, dk=dv=8).

Sharding: 8 cores = (batch b, head-half hh).  Core 2b+hh computes causal
attention for batch b over heads [4hh, 4hh+4) for all 2048 queries and
returns the UNNORMALIZED attention accumulator per chunk: rows 32h hold
the softmax denominator (ones column in V), rows 32h+1..32h+9 the
numerator (exp(s) @ V_h).  The host divides, concatenates heads, and
applies the output projection + bias in f32 (cheap: 0.5 GFLOP total).

Device layout notes (v3 — host-finish + paired projections):
  - Host feeds query/key/value TRANSPOSED ([E, S]) and cast to bf16 so
    projections use them directly as matmul operands at 1 cycle/row.
  - Projection weights are host-packed bf16 "spread" layouts: Q/K outputs
    land at partitions {32h+d}; V outputs at columns {9h+d} with a ones
    column per head at 9h+0 accumulating the softmax denominator.
  - The PE power throttle caps each matmul stream at ~0.78 cols/ns, but
    two tile-disjoint matmuls co-stream at full rate each.  Scores pair
    via row-tiling (tile_position=(32h, 0), K=8); A@V pairs via
    col-tiling (tile_position=(0, 32h), M=9); Q/K projections are split
    into two co-streaming halves (stationary [128, 64] at PE column
    tiles 0 and 64) so they no longer run at half rate.
  - Scores are computed transposed ([t, q]); exp outputs bf16 tiles that
    feed the A@V matmul as the moving operand (V stationary).  The causal
    mask is applied AFTER exp as a bf16 0/1 multiply (keeps the
    scores->exp chain free of DVE hops).
  - Emission is software-pipelined: scores(tb+1) are emitted before
    AV(tb); projections are diced into <=0.7us closures drained via a
    work FIFO, one or two per t-block slot, with chunk boundaries
    prefetching the next chunk's first scores ahead of the final AV
    batch.  Each chunk ends with a DVE copy of the raw AV accumulator
    PSUM->SBUF and a DMA of the [128, 512] f32 block to DRAM.
"""

import math

import numpy as np

B, S, E, H = 4, 2048, 512, 8
DK_H = DV_H = 8
NCORES = 8
HPC = H // 2  # heads per core = 4
SCALE = 1.0 / math.sqrt(DK_H)
NQC = S // 512  # q chunks of 512
NTB = S // 128  # t blocks of 128
ECH = E // 128  # e chunks of 128

_cache: dict = {}


def _apply_tile_patch():
    """walrus in this image allows only one sync-wait per Drain; split the
    TileContext tail drain's waits across a chain of drains."""
    import concourse.mybir as mybir
    from concourse import tile
    from concourse.vector_clock import ScopedClock

    if getattr(tile.TileContext._drain_and_barrier, "_split_patch", False):
        return

    def _drain_and_barrier_split(self, tick_clock, wait_clock):
        drain_inst = self.nc.sync.drain()
        wait_clock.add_sem_waits(
            drain_inst.ins, ScopedClock({None: tick_clock.global_clock})
        )
        si = drain_inst.ins.sync_info
        if si is not None and si.on_wait and len(si.on_wait) > 1:
            waits = list(si.on_wait)
            si.on_wait = waits[:1]
            for entry in waits[1:]:
                extra = self.nc.sync.drain()
                extra.ins.sync_info = mybir.SyncInfo(on_wait=[entry], on_update=[])
        self.nc.all_engine_barrier()
        assert self.sems is not None
        popped = self.nc._tile_sem_poison_stack.pop()
        assert popped is self._sem_poison
        self.nc.clear_and_free_semaphores(list(self.sems.allocated().values()))
        self.nc.all_engine_barrier()

    _drain_and_barrier_split._split_patch = True
    tile.TileContext._drain_and_barrier = _drain_and_barrier_split


def _split_multi_waits(nc):
    """walrus in this image allows only one sync-wait per instruction;
    move excess waits onto single-wait NOPs inserted just before."""
    import concourse.mybir as mybir

    for blk in nc.m.functions[0].blocks:
        out = []
        for inst in blk.instructions:
            si = getattr(inst, "sync_info", None)
            if si is not None and si.on_wait and len(si.on_wait) > 1:
                waits = list(si.on_wait)
                for i, entry in enumerate(waits[:-1]):
                    out.append(
                        mybir.InstNoOp(
                            name=f"{inst.name}_w{i}",
                            engine=inst.engine,
                            ins=[],
                            outs=[],
                            bass_nofuse=True,
                            sync_info=mybir.SyncInfo(
                                on_wait=[entry], on_update=[]
                            ),
                        )
                    )
                si.on_wait = waits[-1:]
            out.append(inst)
        blk.instructions = out


def _build():
    import concourse.bass as bassmod
    import concourse.mybir as mybir
    from concourse import tile

    _apply_tile_patch()
    f32 = mybir.dt.float32
    bf16 = mybir.dt.bfloat16
    Exp = mybir.ActivationFunctionType.Exp

    nc = bassmod.Bass()
    qT = nc.declare_dram_parameter("qT", [E, S], bf16, isOutput=False)
    kT = nc.declare_dram_parameter("kT", [E, S], bf16, isOutput=False)
    vT = nc.declare_dram_parameter("vT", [E, S], bf16, isOutput=False)
    # weights host-packed partition-major so their DMAs are contiguous
    wq = nc.declare_dram_parameter("wq", [128, ECH * 128], bf16, isOutput=False)
    wk = nc.declare_dram_parameter("wk", [128, ECH * 128], bf16, isOutput=False)
    wv = nc.declare_dram_parameter("wv", [128, ECH * HPC * 9], bf16,
                                   isOutput=False)
    msk = nc.declare_dram_parameter("msk", [128, 2 * 128], bf16, isOutput=False)
    out = nc.declare_dram_parameter("out", [128, S], f32, isOutput=True)

    with tile.TileContext(nc) as tc:
        with (
            tc.tile_pool(name="singles", bufs=1) as singles,
            tc.tile_pool(name="loads", bufs=12) as loads,
            tc.tile_pool(name="abuf", bufs=12) as abuf,
            tc.tile_pool(name="outs", bufs=2) as outs,
            tc.tile_pool(name="ps_sc", bufs=2, space="PSUM") as ps_sc,
            tc.tile_pool(name="ps_av", bufs=2, space="PSUM") as ps_av,
            tc.tile_pool(name="ps_misc", bufs=2, space="PSUM") as ps_misc,
        ):
            # ---- resident tensors -------------------------------------
            wq_sb = singles.tile([128, ECH, 128], bf16, tag="wq")
            wk_sb = singles.tile([128, ECH, 128], bf16, tag="wk")
            wv_sb = singles.tile([128, ECH, HPC * 9], bf16, tag="wv")
            msk_sb = singles.tile([128, 2, 128], bf16, tag="msk")
            # startup: K path fully on the Sync DMA queue, Q path (incl. its
            # weight) on GpSimd, so both projection chains run in parallel;
            # wv/msk are deferred until after the q tiles are queued
            nc.gpsimd.dma_start(out=wq_sb, in_=wq.rearrange("p (c m) -> p c m", c=ECH))
            nc.sync.dma_start(out=wk_sb, in_=wk.rearrange("p (c m) -> p c m", c=ECH))

            def late_weights():
                nc.gpsimd.dma_start(
                    out=wv_sb, in_=wv.rearrange("p (c m) -> p c m", c=ECH)
                )
                nc.gpsimd.dma_start(
                    out=msk_sb, in_=msk.rearrange("p (g n) -> p g n", g=2)
                )

            # per-chunk projected tensors (separate tiles so the tile
            # dependency tracker never serializes chunk c's reads against
            # chunk c+2's writes)
            KT_t = [
                singles.tile([128, 512], bf16, tag=f"KT{c}", name=f"KT{c}")
                for c in range(NQC)
            ]
            QT_t = [
                singles.tile([128, 512], bf16, tag=f"QT{c}", name=f"QT{c}")
                for c in range(NQC)
            ]
            V_t = [
                singles.tile([128, 4, HPC, 9], bf16, tag=f"V{c}", name=f"V{c}")
                for c in range(NQC)
            ]

            ones9 = singles.tile([128, 9], bf16, tag="ones9")
            nc.vector.memset(ones9, 1.0)
            for c in range(NQC):
                nc.vector.memset(V_t[c][:, :, :, 0:1], 1.0)

            dmaq = [nc.sync, nc.gpsimd]

            def proj_pieces(c, lo_first=False):
                """Q/K/V projections for chunk c as dicts of emission
                closures (<=0.7us of tensor work each) so pieces fit a
                slot's tensor headroom without starving the exp pipeline.
                With lo_first, the q/k pieces emit only the [0:64] cast
                (heads 0-1); the [64:128] casts are exposed as "q_hi" /
                "k_hi" closures so the first score group can start before
                the second half is cast."""
                cs = slice(c * 512, (c + 1) * 512)
                st = {}

                def dma_in(src, key, n):
                    tiles = []
                    for e in range(ECH):
                        if c == 0 and key in ("k", "q"):
                            # startup: fan out across 3 queues (scalar is
                            # idle until the first scores land) so the
                            # tiles arrive sooner
                            q = [dmaq[0 if key == "k" else 1],
                                 nc.scalar][e // 2]
                        else:
                            q = dmaq[(e + n) % 2]
                        t = loads.tile([128, 512], bf16, tag="ld",
                                       name=f"{key}{c}_{e}")
                        q.dma_start(
                            out=t, in_=src[e * 128:(e + 1) * 128, cs]
                        )
                        tiles.append(t)
                    st[key] = tiles

                def cast_half(key, dst, g):
                    nc.vector.tensor_copy(
                        dst[64 * g:64 * g + 64, :],
                        st[key + "ps"][64 * g:64 * g + 64, :],
                    )

                st["cast"] = cast_half

                def qk_mm(key, w_sb, dst, e, casts=(0, 1)):
                    if e == 0:
                        st[key + "ps"] = ps_misc.tile(
                            [128, 512], f32, tag="ps", name=f"{key}ps{c}"
                        )
                    if c == 0 and key in ("q", "k"):
                        # startup: nothing else is in flight to co-stream
                        # with, so split into two column-tile halves that
                        # pair with each other (halves the wall time)
                        for g in range(2):
                            nc.tensor.matmul(
                                st[key + "ps"][64 * g:64 * g + 64, :],
                                w_sb[:, e, 64 * g:64 * g + 64],
                                st[key][e][:, :],
                                start=(e == 0), stop=(e == ECH - 1),
                                tile_position=(0, 64 * g),
                            )
                    else:
                        # steady state: solo full-width matmul — splitting
                        # is wall-neutral for the pair itself but blocks
                        # BOTH PE stream lanes; a solo leaves the second
                        # lane free for score/AV pairs to ride
                        nc.tensor.matmul(
                            st[key + "ps"], w_sb[:, e, :], st[key][e][:, :],
                            start=(e == 0), stop=(e == ECH - 1),
                        )
                    if e == ECH - 1:
                        # split the cast so the first score batch (heads
                        # 0-1, partitions < 64) gates on the first half
                        for g in casts:
                            cast_half(key, dst, g)

                def v_mm(tb, half):
                    if half == 0:
                        st[f"vps{tb}"] = ps_misc.tile(
                            [128, HPC * 9], f32, tag="ps", name=f"vps{c}_{tb}"
                        )
                    for e in (0, 1) if half == 0 else (2, 3):
                        nc.tensor.matmul(
                            st[f"vps{tb}"],
                            st["v"][e][:, tb * 128:(tb + 1) * 128],
                            wv_sb[:, e, :],
                            start=(e == 0), stop=(e == ECH - 1),
                        )
                    if half == 1:
                        dst = V_t[c][:, tb, :, 1:9]
                        src = st[f"vps{tb}"].rearrange(
                            "p (h n) -> p h n", n=9
                        )[:, :, 1:9]
                        nc.vector.tensor_copy(dst, src)

                qkcasts = (0,) if lo_first else (0, 1)
                return {
                    "q": [
                        lambda e=e: (
                            dma_in(qT, "q", 0) if e == 0 else None,
                            qk_mm("q", wq_sb, QT_t[c], e, qkcasts),
                        )
                        for e in range(ECH)
                    ],
                    "k": [
                        lambda e=e: (
                            dma_in(kT, "k", 1) if e == 0 else None,
                            qk_mm("k", wk_sb, KT_t[c], e, qkcasts),
                        )
                        for e in range(ECH)
                    ],
                    "q_hi": lambda: cast_half("q", QT_t[c], 1),
                    "k_hi": lambda: cast_half("k", KT_t[c], 1),
                    "v": [
                        lambda tb=tb, half=half: (
                            dma_in(vT, "v", 0)
                            if (tb == 0 and half == 0) else None,
                            v_mm(tb, half),
                        )
                        for tb in range(4)
                        for half in range(2)
                    ],
                }

            # ---- attention, software-pipelined ------------------------
            def emit_sa(c, tb, gs=(0, 1), ats=None):
                """Scores + mask + exp for (chunk c, t-block tb), head
                groups `gs` (pass ats back in to finish a partial block)."""
                d = 128 * tb - 512 * c  # diagonal offset within chunk
                vstart = max(d, 0)
                if ats is None:
                    ats = [None, None]
                scg = {}
                # emit ALL score matmuls before the activations: the four
                # row-disjoint tiles can co-stream wider than pairwise when
                # they queue together
                for g in gs:
                    scg[g] = ps_sc.tile([128, 2, 512], f32, tag="sc",
                                        name=f"sc{c}_{tb}_{g}")
                    ats[g] = abuf.tile([128, 2, 512], bf16, tag="a",
                                       name=f"a{c}_{tb}_{g}")
                    for j in range(2):
                        h = 2 * g + j
                        nc.tensor.matmul(
                            scg[g][:, j, vstart:512],
                            KT_t[tb // 4][32 * h:32 * h + 8,
                                          (tb % 4) * 128:(tb % 4 + 1) * 128],
                            QT_t[c][32 * h:32 * h + 8, vstart:512],
                            start=True, stop=True,
                            tile_position=(32 * h, 0),
                        )
                for g in gs:
                    nc.scalar.activation(
                        ats[g][:, :, vstart:512], scg[g][:, :, vstart:512],
                        Exp, scale=SCALE,
                    )
                    if d >= 0:
                        # zero the upper triangle AFTER exp (bf16 SBUF mul
                        # runs in the DVE 2x mode and keeps the scores->exp
                        # chain free of DVE hops)
                        nc.vector.tensor_mul(
                            ats[g][:, :, d:d + 128],
                            ats[g][:, :, d:d + 128],
                            msk_sb,
                        )
                return ats, vstart

            # prefix: chunk 0's K/Q projections inline, first scores/exp,
            # then chunk 0's first V block and chunk 1's Q/K — all during
            # the act-idle startup window
            sa_pre = {}  # (c, tb) -> (ats, vstart) emitted ahead of its chunk
            p0 = proj_pieces(0, lo_first=True)
            pp = {c: proj_pieces(c) for c in range(1, NQC)}
            for f in p0["k"]:
                f()
            for f in p0["q"]:
                f()
            late_weights()
            # preload the exp activation table (1.3us) on the scalar queue
            # AFTER its startup DMA issues, still ahead of the first scores
            warm = abuf.tile([128, 2], bf16, tag="warm")
            nc.scalar.activation(warm, ones9[:, 0:2], Exp, scale=1.0)
            # first score group fires on the lo casts alone; the hi casts
            # run on DVE while the act engine chews on group 0
            ats00, vs00 = emit_sa(0, 0, gs=(0,))
            p0["k_hi"]()
            p0["q_hi"]()
            emit_sa(0, 0, gs=(1,), ats=ats00)
            sa_pre[(0, 0)] = (ats00, vs00)
            # keep the act pipeline primed while the first pieces drain
            # (before the v pieces: their matmuls wait on the late vT DMAs
            # and would head-of-line-block the in-order PE queue)
            sa_pre[(0, 1)] = emit_sa(0, 1)
            p0["v"][0]()
            p0["v"][1]()

            work = []  # FIFO of deferred emission closures
            avs = {}
            for c in range(NQC):
                av = avs[c] = ps_av.tile([128, 512], f32, tag="av",
                                         name=f"av{c}")
                nc.vector.memset(av, 0.0)
                ntb = 4 * (c + 1)

                if c == 0:
                    work.extend(p0["v"][2:8])  # V1-3(0)
                    work.extend(pp[1]["q"])
                else:
                    # k(c) drains in this chunk's early slots — KT_t[c] is
                    # first read by this chunk's t-block 4c, i.e. slot 4c
                    work.extend(pp[c]["k"])
                    work.extend(pp[c]["v"])
                    if c + 1 < NQC:
                        work.extend(pp[c + 1]["q"])
                    else:
                        work.extend([None, None, None])

                pend = None  # (tb, ats, vstart) waiting for its AV matmuls
                for tb in range(ntb):
                    if (c, tb) in sa_pre:
                        ats, vstart = sa_pre.pop((c, tb))
                    else:
                        ats, vstart = emit_sa(c, tb)
                    # AV for the PREVIOUS tb — keeps exp ahead of the PE
                    if pend is not None:
                        ptb, pats, pvs = pend
                        for h in range(HPC):
                            g, j = divmod(h, 2)
                            nc.tensor.matmul(
                                av[32 * h:32 * h + 9, pvs:512],
                                V_t[ptb // 4][:, ptb % 4, h, :],
                                pats[g][:, j, pvs:512],
                                start=(ptb == 0), stop=False,
                                tile_position=(0, 32 * h),
                            )
                    pend = (tb, ats, vstart)
                    # drain the FIFO evenly across this chunk's slots,
                    # holding back the first 2 slots (c>0) so the boundary
                    # scores are never stuck behind pieces
                    hold = 0 if c == 0 else 2
                    if tb >= hold:
                        npop = -(-len(work) // (ntb - tb))  # ceil
                        for _ in range(npop):
                            piece = work.pop(0)
                            if piece is not None:
                                piece()
                # prefetch the next chunk's first TWO score/exp groups
                # ahead of the final AV batch so the act pipeline never
                # drains at the boundary (the second group would otherwise
                # queue behind the final-AV matmuls)
                if c + 1 < NQC:
                    sa_pre[(c + 1, 0)] = emit_sa(c + 1, 0)
                    sa_pre[(c + 1, 1)] = emit_sa(c + 1, 1)
                av_sb = outs.tile([128, 512], f32, tag="avsb",
                                  name=f"avsb{c}")
                if c == NQC - 1:
                    # tail: columns [0, pvs) are final once AV(ntb-2) is in
                    # (the last block only touches [pvs, 512)) — ship them
                    # now so the drain doesn't wait on the big transfer
                    lo = pend[2]  # = vstart of the final t-block
                    nc.vector.tensor_copy(av_sb[:, 0:lo], av[:, 0:lo])
                    dmaq[c % 2].dma_start(
                        out=out[:, c * 512:c * 512 + lo], in_=av_sb[:, 0:lo]
                    )
                else:
                    lo = 0
                ptb, pats, pvs = pend
                for h in range(HPC):
                    g, j = divmod(h, 2)
                    nc.tensor.matmul(
                        av[32 * h:32 * h + 9, pvs:512],
                        V_t[ptb // 4][:, ptb % 4, h, :],
                        pats[g][:, j, pvs:512],
                        start=(ptb == 0), stop=True,
                        tile_position=(0, 32 * h),
                    )
                # ship the raw accumulator (numerators + denominators):
                # PSUM -> SBUF copy on DVE, then DMA out.  Emitted here
                # (before the next chunk's av memset in DVE program order)
                # so the bufs=2 rotation can never deadlock.
                nc.vector.tensor_copy(av_sb[:, lo:512], av[:, lo:512])
                dmaq[c % 2].dma_start(
                    out=out[:, c * 512 + lo:(c + 1) * 512],
                    in_=av_sb[:, lo:512]
                )
    _split_multi_waits(nc)
    return nc


def _prep_inputs(query, key, value, Wq, Wk, Wv):
    """Build the 8 per-core input maps (host-side sharding/layout)."""
    import ml_dtypes

    bf16 = np.dtype(ml_dtypes.bfloat16)
    qTs = [np.ascontiguousarray(query[b].T).astype(bf16) for b in range(B)]
    kTs = [np.ascontiguousarray(key[b].T).astype(bf16) for b in range(B)]
    vTs = [np.ascontiguousarray(value[b].T).astype(bf16) for b in range(B)]

    mask = np.where(
        np.arange(128)[:, None] <= np.arange(128)[None, :], 1.0, 0.0
    ).astype(np.float32)
    msk2 = np.ascontiguousarray(np.tile(mask, (1, 2))).astype(bf16)

    in_maps = []
    for core in range(NCORES):
        b, hh = divmod(core, 2)
        wq_p = np.zeros((E, 128), np.float32)
        wk_p = np.zeros((E, 128), np.float32)
        wv_p = np.zeros((E, HPC * 9), np.float32)
        for h in range(HPC):
            g = 4 * hh + h
            wq_p[:, 32 * h:32 * h + 8] = Wq[g]
            wk_p[:, 32 * h:32 * h + 8] = Wk[g]
            wv_p[:, 9 * h + 1:9 * h + 9] = Wv[g]
        def pack(w):  # [E, M] -> [128, ECH*M] partition-major
            m = w.shape[1]
            return np.ascontiguousarray(
                w.reshape(ECH, 128, m).transpose(1, 0, 2).reshape(128, ECH * m)
            )

        in_maps.append(
            {
                "qT": qTs[b], "kT": kTs[b], "vT": vTs[b],
                "wq": pack(wq_p).astype(bf16), "wk": pack(wk_p).astype(bf16),
                "wv": pack(wv_p).astype(bf16),
                "msk": msk2,
            }
        )
    return in_maps


def _reference_numpy(query, key, value, padding_mask, decoder_mask,
                     Wq, Wk, Wv, Wo, bo):
    """Fallback (non-default masks): plain numpy replica of the reference."""
    q = np.einsum("bse,hed->bhsd", query, Wq)
    k = np.einsum("bse,hed->bhsd", key, Wk)
    v = np.einsum("bse,hed->bhsd", value, Wv)
    s = np.einsum("bhsd,bhtd->bhst", q, k)
    if decoder_mask:
        tril = np.tril(s)
        s = np.where(tril == 0.0, -np.inf, s)
    s = np.where(padding_mask[:, None, :, :], s, -np.inf)
    s = s / np.sqrt(np.float32(DK_H))
    m = np.max(s, axis=-1, keepdims=True)
    e = np.exp(s - m)
    a = e / np.sum(e, axis=-1, keepdims=True)
    o = np.einsum("bhst,bhtd->bhsd", a, v)
    o = o.transpose(0, 2, 1, 3).reshape(o.shape[0], o.shape[2], H * DV_H)
    return (o @ Wo + bo).astype(np.float32)


def kernel(query, key, value, padding_mask, decoder_mask, Wq, Wk, Wv, Wo, bo,
           **run_kwargs):
    query = np.asarray(query, np.float32)
    key = np.asarray(key, np.float32)
    value = np.asarray(value, np.float32)
    Wq = np.asarray(Wq, np.float32)
    Wk = np.asarray(Wk, np.float32)
    Wv = np.asarray(Wv, np.float32)
    Wo = np.asarray(Wo, np.float32)
    bo = np.asarray(bo, np.float32)
    pm = np.asarray(padding_mask)
    dm = int(np.asarray(decoder_mask))

    if not bool(pm.all()) or not dm:
        return _reference_numpy(
            query, key, value, pm.astype(bool), dm, Wq, Wk, Wv, Wo, bo
        )

    from concourse.bass_utils import run_bass_kernel_spmd

    if "nc" not in _cache:
        _cache["nc"] = _build()
    nc = _cache["nc"]

    in_maps = _prep_inputs(query, key, value, Wq, Wk, Wv)
    res = run_bass_kernel_spmd(nc, in_maps, list(range(NCORES)), **run_kwargs)

    # host finish: divide numerators by denominators, concatenate heads,
    # apply the output projection + bias (all f32)
    outp = np.empty((B, S, E), np.float32)
    ov = np.empty((S, H * DV_H), np.float32)
    for b in range(B):
        for hh in range(2):
            r = res.results[2 * b + hh]["out"]  # [128, S] f32
            for h in range(HPC):
                g = 4 * hh + h
                den = r[32 * h]
                num = r[32 * h + 1:32 * h + 9]
                ov[:, 8 * g:8 * g + 8] = (num / den).T
        outp[b] = ov @ Wo + bo
    if run_kwargs:
        kernel.last_result = res
    return outp


# revision 21
# speedup vs baseline: 1.0132x; 1.0132x over previous
"""Bass/Trainium2 kernel for nn_MultiHeadAttention (B=4, S=2048, E=512, H=8, dk=dv=8).

Sharding: 8 cores = (batch b, head-half hh).  Core 2b+hh computes causal
attention for batch b over heads [4hh, 4hh+4) for all 2048 queries and
returns the UNNORMALIZED attention accumulator per chunk: rows 32h hold
the softmax denominator (ones column in V), rows 32h+1..32h+9 the
numerator (exp(s) @ V_h).  The host divides, concatenates heads, and
applies the output projection + bias in f32 (cheap: 0.5 GFLOP total).

Device layout notes (v3 — host-finish + paired projections):
  - Host feeds query/key/value TRANSPOSED ([E, S]) and cast to bf16 so
    projections use them directly as matmul operands at 1 cycle/row.
  - Projection weights are host-packed bf16 "spread" layouts: Q/K outputs
    land at partitions {32h+d}; V outputs at columns {9h+d} with a ones
    column per head at 9h+0 accumulating the softmax denominator.
  - The PE power throttle caps each matmul stream at ~0.78 cols/ns, but
    two tile-disjoint matmuls co-stream at full rate each.  Scores pair
    via row-tiling (tile_position=(32h, 0), K=8); A@V pairs via
    col-tiling (tile_position=(0, 32h), M=9); Q/K projections are split
    into two co-streaming halves (stationary [128, 64] at PE column
    tiles 0 and 64) so they no longer run at half rate.
  - Scores are computed transposed ([t, q]); exp outputs bf16 tiles that
    feed the A@V matmul as the moving operand (V stationary).  The causal
    mask is applied AFTER exp as a bf16 0/1 multiply (keeps the
    scores->exp chain free of DVE hops).
  - Emission is software-pipelined: scores(tb+1) are emitted before
    AV(tb); projections are diced into <=0.7us closures drained via a
    work FIFO, one or two per t-block slot, with chunk boundaries
    prefetching the next chunk's first scores ahead of the final AV
    batch.  Each chunk ends with a DVE copy of the raw AV accumulator
    PSUM->SBUF and a DMA of the [128, 512] f32 block to DRAM.
"""

import math

import numpy as np

B, S, E, H = 4, 2048, 512, 8
DK_H = DV_H = 8
NCORES = 8
HPC = H // 2  # heads per core = 4
SCALE = 1.0 / math.sqrt(DK_H)
NQC = S // 512  # q chunks of 512
NTB = S // 128  # t blocks of 128
ECH = E // 128  # e chunks of 128

_cache: dict = {}


def _apply_tile_patch():
    """walrus in this image allows only one sync-wait per Drain; split the
    TileContext tail drain's waits across a chain of drains."""
    import concourse.mybir as mybir
    from concourse import tile
    from concourse.vector_clock import ScopedClock

    if getattr(tile.TileContext._drain_and_barrier, "_split_patch", False):
        return

    def _drain_and_barrier_split(self, tick_clock, wait_clock):
        drain_inst = self.nc.sync.drain()
        wait_clock.add_sem_waits(
            drain_inst.ins, ScopedClock({None: tick_clock.global_clock})
        )
        si = drain_inst.ins.sync_info
        if si is not None and si.on_wait and len(si.on_wait) > 1:
            waits = list(si.on_wait)
            si.on_wait = waits[:1]
            for entry in waits[1:]:
                extra = self.nc.sync.drain()
                extra.ins.sync_info = mybir.SyncInfo(on_wait=[entry], on_update=[])
        self.nc.all_engine_barrier()
        assert self.sems is not None
        popped = self.nc._tile_sem_poison_stack.pop()
        assert popped is self._sem_poison
        self.nc.clear_and_free_semaphores(list(self.sems.allocated().values()))
        self.nc.all_engine_barrier()

    _drain_and_barrier_split._split_patch = True
    tile.TileContext._drain_and_barrier = _drain_and_barrier_split


def _split_multi_waits(nc):
    """walrus in this image allows only one sync-wait per instruction;
    move excess waits onto single-wait NOPs inserted just before."""
    import concourse.mybir as mybir

    for blk in nc.m.functions[0].blocks:
        out = []
        for inst in blk.instructions:
            si = getattr(inst, "sync_info", None)
            if si is not None and si.on_wait and len(si.on_wait) > 1:
                waits = list(si.on_wait)
                for i, entry in enumerate(waits[:-1]):
                    out.append(
                        mybir.InstNoOp(
                            name=f"{inst.name}_w{i}",
                            engine=inst.engine,
                            ins=[],
                            outs=[],
                            bass_nofuse=True,
                            sync_info=mybir.SyncInfo(
                                on_wait=[entry], on_update=[]
                            ),
                        )
                    )
                si.on_wait = waits[-1:]
            out.append(inst)
        blk.instructions = out


def _build():
    import concourse.bass as bassmod
    import concourse.mybir as mybir
    from concourse import tile

    _apply_tile_patch()
    f32 = mybir.dt.float32
    bf16 = mybir.dt.bfloat16
    Exp = mybir.ActivationFunctionType.Exp

    nc = bassmod.Bass()
    qT = nc.declare_dram_parameter("qT", [E, S], bf16, isOutput=False)
    kT = nc.declare_dram_parameter("kT", [E, S], bf16, isOutput=False)
    vT = nc.declare_dram_parameter("vT", [E, S], bf16, isOutput=False)
    # weights host-packed partition-major so their DMAs are contiguous
    wq = nc.declare_dram_parameter("wq", [128, ECH * 128], bf16, isOutput=False)
    wk = nc.declare_dram_parameter("wk", [128, ECH * 128], bf16, isOutput=False)
    wv = nc.declare_dram_parameter("wv", [128, ECH * HPC * 9], bf16,
                                   isOutput=False)
    msk = nc.declare_dram_parameter("msk", [128, 2 * 128], bf16, isOutput=False)
    out = nc.declare_dram_parameter("out", [128, S], f32, isOutput=True)

    with tile.TileContext(nc) as tc:
        with (
            tc.tile_pool(name="singles", bufs=1) as singles,
            tc.tile_pool(name="loads", bufs=12) as loads,
            tc.tile_pool(name="abuf", bufs=12) as abuf,
            tc.tile_pool(name="outs", bufs=2) as outs,
            tc.tile_pool(name="ps_sc", bufs=2, space="PSUM") as ps_sc,
            tc.tile_pool(name="ps_av", bufs=2, space="PSUM") as ps_av,
            tc.tile_pool(name="ps_misc", bufs=2, space="PSUM") as ps_misc,
        ):
            # ---- resident tensors -------------------------------------
            wq_sb = singles.tile([128, ECH, 128], bf16, tag="wq")
            wk_sb = singles.tile([128, ECH, 128], bf16, tag="wk")
            wv_sb = singles.tile([128, ECH, HPC * 9], bf16, tag="wv")
            msk_sb = singles.tile([128, 2, 128], bf16, tag="msk")
            # startup: K path fully on the Sync DMA queue, Q path (incl. its
            # weight) on GpSimd, so both projection chains run in parallel;
            # wv/msk are deferred until after the q tiles are queued
            nc.gpsimd.dma_start(out=wq_sb, in_=wq.rearrange("p (c m) -> p c m", c=ECH))
            nc.sync.dma_start(out=wk_sb, in_=wk.rearrange("p (c m) -> p c m", c=ECH))

            def late_weights():
                nc.gpsimd.dma_start(
                    out=wv_sb, in_=wv.rearrange("p (c m) -> p c m", c=ECH)
                )
                nc.gpsimd.dma_start(
                    out=msk_sb, in_=msk.rearrange("p (g n) -> p g n", g=2)
                )

            # per-chunk projected tensors (separate tiles so the tile
            # dependency tracker never serializes chunk c's reads against
            # chunk c+2's writes)
            KT_t = [
                singles.tile([128, 512], bf16, tag=f"KT{c}", name=f"KT{c}")
                for c in range(NQC)
            ]
            QT_t = [
                singles.tile([128, 512], bf16, tag=f"QT{c}", name=f"QT{c}")
                for c in range(NQC)
            ]
            V_t = [
                singles.tile([128, 4, HPC, 9], bf16, tag=f"V{c}", name=f"V{c}")
                for c in range(NQC)
            ]

            ones9 = singles.tile([128, 9], bf16, tag="ones9")
            nc.vector.memset(ones9, 1.0)
            for c in range(NQC):
                nc.vector.memset(V_t[c][:, :, :, 0:1], 1.0)

            dmaq = [nc.sync, nc.gpsimd]

            def proj_pieces(c, lo_first=False):
                """Q/K/V projections for chunk c as dicts of emission
                closures (<=0.7us of tensor work each) so pieces fit a
                slot's tensor headroom without starving the exp pipeline.
                With lo_first, the q/k pieces emit only the [0:64] cast
                (heads 0-1); the [64:128] casts are exposed as "q_hi" /
                "k_hi" closures so the first score group can start before
                the second half is cast."""
                cs = slice(c * 512, (c + 1) * 512)
                st = {}

                def dma_in(src, key, n):
                    tiles = []
                    for e in range(ECH):
                        if c == 0 and key in ("k", "q"):
                            q = dmaq[0 if key == "k" else 1]
                        else:
                            q = dmaq[(e + n) % 2]
                        t = loads.tile([128, 512], bf16, tag="ld",
                                       name=f"{key}{c}_{e}")
                        q.dma_start(
                            out=t, in_=src[e * 128:(e + 1) * 128, cs]
                        )
                        tiles.append(t)
                    st[key] = tiles

                def cast_half(key, dst, g):
                    nc.vector.tensor_copy(
                        dst[64 * g:64 * g + 64, :],
                        st[key + "ps"][64 * g:64 * g + 64, :],
                    )

                st["cast"] = cast_half

                def qk_mm(key, w_sb, dst, e, casts=(0, 1)):
                    if e == 0:
                        st[key + "ps"] = ps_misc.tile(
                            [128, 512], f32, tag="ps", name=f"{key}ps{c}"
                        )
                    if c == 0 and key in ("q", "k"):
                        # startup: nothing else is in flight to co-stream
                        # with, so split into two column-tile halves that
                        # pair with each other (halves the wall time)
                        for g in range(2):
                            nc.tensor.matmul(
                                st[key + "ps"][64 * g:64 * g + 64, :],
                                w_sb[:, e, 64 * g:64 * g + 64],
                                st[key][e][:, :],
                                start=(e == 0), stop=(e == ECH - 1),
                                tile_position=(0, 64 * g),
                            )
                    else:
                        # steady state: solo full-width matmul — splitting
                        # is wall-neutral for the pair itself but blocks
                        # BOTH PE stream lanes; a solo leaves the second
                        # lane free for score/AV pairs to ride
                        nc.tensor.matmul(
                            st[key + "ps"], w_sb[:, e, :], st[key][e][:, :],
                            start=(e == 0), stop=(e == ECH - 1),
                        )
                    if e == ECH - 1:
                        # split the cast so the first score batch (heads
                        # 0-1, partitions < 64) gates on the first half
                        for g in casts:
                            cast_half(key, dst, g)

                def v_mm(tb, half):
                    if half == 0:
                        st[f"vps{tb}"] = ps_misc.tile(
                            [128, HPC * 9], f32, tag="ps", name=f"vps{c}_{tb}"
                        )
                    for e in (0, 1) if half == 0 else (2, 3):
                        nc.tensor.matmul(
                            st[f"vps{tb}"],
                            st["v"][e][:, tb * 128:(tb + 1) * 128],
                            wv_sb[:, e, :],
                            start=(e == 0), stop=(e == ECH - 1),
                        )
                    if half == 1:
                        dst = V_t[c][:, tb, :, 1:9]
                        src = st[f"vps{tb}"].rearrange(
                            "p (h n) -> p h n", n=9
                        )[:, :, 1:9]
                        nc.vector.tensor_copy(dst, src)

                qkcasts = (0,) if lo_first else (0, 1)
                return {
                    "q": [
                        lambda e=e: (
                            dma_in(qT, "q", 0) if e == 0 else None,
                            qk_mm("q", wq_sb, QT_t[c], e, qkcasts),
                        )
                        for e in range(ECH)
                    ],
                    "k": [
                        lambda e=e: (
                            dma_in(kT, "k", 1) if e == 0 else None,
                            qk_mm("k", wk_sb, KT_t[c], e, qkcasts),
                        )
                        for e in range(ECH)
                    ],
                    "q_hi": lambda: cast_half("q", QT_t[c], 1),
                    "k_hi": lambda: cast_half("k", KT_t[c], 1),
                    "v": [
                        lambda tb=tb, half=half: (
                            dma_in(vT, "v", 0)
                            if (tb == 0 and half == 0) else None,
                            v_mm(tb, half),
                        )
                        for tb in range(4)
                        for half in range(2)
                    ],
                }

            # ---- attention, software-pipelined ------------------------
            def emit_sa(c, tb, gs=(0, 1), ats=None):
                """Scores + mask + exp for (chunk c, t-block tb), head
                groups `gs` (pass ats back in to finish a partial block)."""
                d = 128 * tb - 512 * c  # diagonal offset within chunk
                vstart = max(d, 0)
                if ats is None:
                    ats = [None, None]
                scg = {}
                # emit ALL score matmuls before the activations: the four
                # row-disjoint tiles can co-stream wider than pairwise when
                # they queue together
                for g in gs:
                    scg[g] = ps_sc.tile([128, 2, 512], f32, tag="sc",
                                        name=f"sc{c}_{tb}_{g}")
                    ats[g] = abuf.tile([128, 2, 512], bf16, tag="a",
                                       name=f"a{c}_{tb}_{g}")
                    for j in range(2):
                        h = 2 * g + j
                        nc.tensor.matmul(
                            scg[g][:, j, vstart:512],
                            KT_t[tb // 4][32 * h:32 * h + 8,
                                          (tb % 4) * 128:(tb % 4 + 1) * 128],
                            QT_t[c][32 * h:32 * h + 8, vstart:512],
                            start=True, stop=True,
                            tile_position=(32 * h, 0),
                        )
                for g in gs:
                    nc.scalar.activation(
                        ats[g][:, :, vstart:512], scg[g][:, :, vstart:512],
                        Exp, scale=SCALE,
                    )
                    if d >= 0:
                        # zero the upper triangle AFTER exp (bf16 SBUF mul
                        # runs in the DVE 2x mode and keeps the scores->exp
                        # chain free of DVE hops)
                        nc.vector.tensor_mul(
                            ats[g][:, :, d:d + 128],
                            ats[g][:, :, d:d + 128],
                            msk_sb,
                        )
                return ats, vstart

            # prefix: chunk 0's K/Q projections inline, first scores/exp,
            # then chunk 0's first V block and chunk 1's Q/K — all during
            # the act-idle startup window
            sa_pre = {}  # (c, tb) -> (ats, vstart) emitted ahead of its chunk
            p0 = proj_pieces(0, lo_first=True)
            pp = {c: proj_pieces(c) for c in range(1, NQC)}
            for f in p0["k"]:
                f()
            for f in p0["q"]:
                f()
            late_weights()
            # preload the exp activation table (1.3us) on the scalar queue
            # AFTER its startup DMA issues, still ahead of the first scores
            warm = abuf.tile([128, 2], bf16, tag="warm")
            nc.scalar.activation(warm, ones9[:, 0:2], Exp, scale=1.0)
            # first score group fires on the lo casts alone; the hi casts
            # run on DVE while the act engine chews on group 0
            ats00, vs00 = emit_sa(0, 0, gs=(0,))
            p0["k_hi"]()
            p0["q_hi"]()
            emit_sa(0, 0, gs=(1,), ats=ats00)
            sa_pre[(0, 0)] = (ats00, vs00)
            # keep the act pipeline primed while the q1/k1 projections and
            # first v pieces drain through the in-order PE queue
            sa_pre[(0, 1)] = emit_sa(0, 1)
            p0["v"][0]()
            p0["v"][1]()
            for f in pp[1]["q"]:
                f()
            for f in pp[1]["k"]:
                f()

            work = []  # FIFO of deferred emission closures
            avs = {}
            for c in range(NQC):
                av = avs[c] = ps_av.tile([128, 512], f32, tag="av",
                                         name=f"av{c}")
                nc.vector.memset(av, 0.0)
                ntb = 4 * (c + 1)

                if c == 0:
                    work.extend(p0["v"][2:8])  # V1-3(0)
                else:
                    if c >= 2:
                        work.extend(pp[c]["k"])
                    work.extend(pp[c]["v"])
                    if c + 1 < NQC:
                        work.extend(pp[c + 1]["q"])
                    else:
                        work.extend([None, None, None])

                pend = None  # (tb, ats, vstart) waiting for its AV matmuls
                for tb in range(ntb):
                    if (c, tb) in sa_pre:
                        ats, vstart = sa_pre.pop((c, tb))
                    else:
                        ats, vstart = emit_sa(c, tb)
                    # AV for the PREVIOUS tb — keeps exp ahead of the PE
                    if pend is not None:
                        ptb, pats, pvs = pend
                        for h in range(HPC):
                            g, j = divmod(h, 2)
                            nc.tensor.matmul(
                                av[32 * h:32 * h + 9, pvs:512],
                                V_t[ptb // 4][:, ptb % 4, h, :],
                                pats[g][:, j, pvs:512],
                                start=(ptb == 0), stop=False,
                                tile_position=(0, 32 * h),
                            )
                    pend = (tb, ats, vstart)
                    # drain the FIFO evenly across this chunk's slots,
                    # holding back the first 2 slots (c>0) so the boundary
                    # scores are never stuck behind pieces
                    hold = 0 if c == 0 else 2
                    if tb >= hold:
                        npop = -(-len(work) // (ntb - tb))  # ceil
                        for _ in range(npop):
                            piece = work.pop(0)
                            if piece is not None:
                                piece()
                # prefetch the next chunk's first TWO score/exp groups
                # ahead of the final AV batch so the act pipeline never
                # drains at the boundary (the second group would otherwise
                # queue behind the final-AV matmuls)
                if c + 1 < NQC:
                    sa_pre[(c + 1, 0)] = emit_sa(c + 1, 0)
                    sa_pre[(c + 1, 1)] = emit_sa(c + 1, 1)
                av_sb = outs.tile([128, 512], f32, tag="avsb",
                                  name=f"avsb{c}")
                if c == NQC - 1:
                    # tail: columns [0, pvs) are final once AV(ntb-2) is in
                    # (the last block only touches [pvs, 512)) — ship them
                    # now so the drain doesn't wait on the big transfer
                    lo = pend[2]  # = vstart of the final t-block
                    nc.vector.tensor_copy(av_sb[:, 0:lo], av[:, 0:lo])
                    dmaq[c % 2].dma_start(
                        out=out[:, c * 512:c * 512 + lo], in_=av_sb[:, 0:lo]
                    )
                else:
                    lo = 0
                ptb, pats, pvs = pend
                for h in range(HPC):
                    g, j = divmod(h, 2)
                    nc.tensor.matmul(
                        av[32 * h:32 * h + 9, pvs:512],
                        V_t[ptb // 4][:, ptb % 4, h, :],
                        pats[g][:, j, pvs:512],
                        start=(ptb == 0), stop=True,
                        tile_position=(0, 32 * h),
                    )
                # ship the raw accumulator (numerators + denominators):
                # PSUM -> SBUF copy on DVE, then DMA out.  Emitted here
                # (before the next chunk's av memset in DVE program order)
                # so the bufs=2 rotation can never deadlock.
                nc.vector.tensor_copy(av_sb[:, lo:512], av[:, lo:512])
                dmaq[c % 2].dma_start(
                    out=out[:, c * 512 + lo:(c + 1) * 512],
                    in_=av_sb[:, lo:512]
                )
    _split_multi_waits(nc)
    return nc


def _prep_inputs(query, key, value, Wq, Wk, Wv):
    """Build the 8 per-core input maps (host-side sharding/layout)."""
    import ml_dtypes

    bf16 = np.dtype(ml_dtypes.bfloat16)
    qTs = [np.ascontiguousarray(query[b].T).astype(bf16) for b in range(B)]
    kTs = [np.ascontiguousarray(key[b].T).astype(bf16) for b in range(B)]
    vTs = [np.ascontiguousarray(value[b].T).astype(bf16) for b in range(B)]

    mask = np.where(
        np.arange(128)[:, None] <= np.arange(128)[None, :], 1.0, 0.0
    ).astype(np.float32)
    msk2 = np.ascontiguousarray(np.tile(mask, (1, 2))).astype(bf16)

    in_maps = []
    for core in range(NCORES):
        b, hh = divmod(core, 2)
        wq_p = np.zeros((E, 128), np.float32)
        wk_p = np.zeros((E, 128), np.float32)
        wv_p = np.zeros((E, HPC * 9), np.float32)
        for h in range(HPC):
            g = 4 * hh + h
            wq_p[:, 32 * h:32 * h + 8] = Wq[g]
            wk_p[:, 32 * h:32 * h + 8] = Wk[g]
            wv_p[:, 9 * h + 1:9 * h + 9] = Wv[g]
        def pack(w):  # [E, M] -> [128, ECH*M] partition-major
            m = w.shape[1]
            return np.ascontiguousarray(
                w.reshape(ECH, 128, m).transpose(1, 0, 2).reshape(128, ECH * m)
            )

        in_maps.append(
            {
                "qT": qTs[b], "kT": kTs[b], "vT": vTs[b],
                "wq": pack(wq_p).astype(bf16), "wk": pack(wk_p).astype(bf16),
                "wv": pack(wv_p).astype(bf16),
                "msk": msk2,
            }
        )
    return in_maps


def _reference_numpy(query, key, value, padding_mask, decoder_mask,
                     Wq, Wk, Wv, Wo, bo):
    """Fallback (non-default masks): plain numpy replica of the reference."""
    q = np.einsum("bse,hed->bhsd", query, Wq)
    k = np.einsum("bse,hed->bhsd", key, Wk)
    v = np.einsum("bse,hed->bhsd", value, Wv)
    s = np.einsum("bhsd,bhtd->bhst", q, k)
    if decoder_mask:
        tril = np.tril(s)
        s = np.where(tril == 0.0, -np.inf, s)
    s = np.where(padding_mask[:, None, :, :], s, -np.inf)
    s = s / np.sqrt(np.float32(DK_H))
    m = np.max(s, axis=-1, keepdims=True)
    e = np.exp(s - m)
    a = e / np.sum(e, axis=-1, keepdims=True)
    o = np.einsum("bhst,bhtd->bhsd", a, v)
    o = o.transpose(0, 2, 1, 3).reshape(o.shape[0], o.shape[2], H * DV_H)
    return (o @ Wo + bo).astype(np.float32)


def kernel(query, key, value, padding_mask, decoder_mask, Wq, Wk, Wv, Wo, bo,
           **run_kwargs):
    query = np.asarray(query, np.float32)
    key = np.asarray(key, np.float32)
    value = np.asarray(value, np.float32)
    Wq = np.asarray(Wq, np.float32)
    Wk = np.asarray(Wk, np.float32)
    Wv = np.asarray(Wv, np.float32)
    Wo = np.asarray(Wo, np.float32)
    bo = np.asarray(bo, np.float32)
    pm = np.asarray(padding_mask)
    dm = int(np.asarray(decoder_mask))

    if not bool(pm.all()) or not dm:
        return _reference_numpy(
            query, key, value, pm.astype(bool), dm, Wq, Wk, Wv, Wo, bo
        )

    from concourse.bass_utils import run_bass_kernel_spmd

    if "nc" not in _cache:
        _cache["nc"] = _build()
    nc = _cache["nc"]

    in_maps = _prep_inputs(query, key, value, Wq, Wk, Wv)
    res = run_bass_kernel_spmd(nc, in_maps, list(range(NCORES)), **run_kwargs)

    # host finish: divide numerators by denominators, concatenate heads,
    # apply the output projection + bias (all f32)
    outp = np.empty((B, S, E), np.float32)
    ov = np.empty((S, H * DV_H), np.float32)
    for b in range(B):
        for hh in range(2):
            r = res.results[2 * b + hh]["out"]  # [128, S] f32
            for h in range(HPC):
                g = 4 * hh + h
                den = r[32 * h]
                num = r[32 * h + 1:32 * h + 9]
                ov[:, 8 * g:8 * g + 8] = (num / den).T
        outp[b] = ov @ Wo + bo
    if run_kwargs:
        kernel.last_result = res
    return outp


# revision 51
# speedup vs baseline: 1.0750x; 1.0610x over previous
"""Bass/Trainium2 kernel for nn_MultiHeadAttention (B=4, S=2048, E=512, H=8, dk=dv=8).

Sharding: 8 cores = (batch b, head-half hh).  Core 2b+hh computes causal
attention for batch b over heads [4hh, 4hh+4) for all 2048 queries and
returns the UNNORMALIZED attention accumulator per chunk: rows 32h hold
the softmax denominator (ones column in V), rows 32h+1..32h+9 the
numerator (exp(s) @ V_h).  The host divides, concatenates heads, and
applies the output projection + bias in f32 (cheap: 0.5 GFLOP total).

Device layout notes (v4 — host-finish, deep score buffering, packed DMA):
  - Host feeds query/key/value as host-packed CONTIGUOUS [128, 512] bf16
    tiles, (chunk, e)-major, so every input DMA is a single 128KB burst
    (the [E, S] view's 1KB-of-4KB strided reads throttled startup).
  - Projection weights are host-packed bf16 "spread" layouts: Q/K outputs
    land at partitions {32h+d}; V outputs at columns {9h+d} with a ones
    column per head at 9h+0 accumulating the softmax denominator.
  - The PE power throttle caps each matmul stream at ~0.78 cols/ns, but
    two tile-disjoint matmuls co-stream at full rate each (measured; >2
    rarely co-streams, and matmul PSUM destinations must be 2KB-bank-
    aligned, which forbids packing 4 quad-issued heads into fewer banks).
    Scores pair via row-tiling (tile_position=(32h, 0), K=8); A@V pairs
    via col-tiling (tile_position=(0, 32h), M=9) and sits at the PE's
    ~205 G elem/s element cap.  Q/K projections run as full-width solos
    in steady state (splitting them only burns the second stream lane);
    chunk 0's run as co-streaming column-tile halves since nothing else
    is in flight at startup.
  - Scores are computed transposed ([t, q]); exp outputs bf16 tiles that
    feed the A@V matmul as the moving operand (V stationary).  The causal
    mask is applied AFTER exp as a bf16 0/1 multiply (keeps the
    scores->exp chain free of DVE hops).
  - PSUM: score tiles [128, 2, 512] f32 x3 bufs (6 banks) so a score
    pair is released three activations back; av accumulator x1 (no
    memset — AV chains start with start=True and the host ignores
    unwritten rows); projection psum x1 (chunk-0's K/Q borrow idle
    score-pool buffers to keep their chains parallel at startup).
  - Emission is software-pipelined: scores(tb+1) are emitted before
    AV(tb); projections are diced into <=0.7us closures drained via a
    work FIFO ordered [k(c), q(c+1), v(c)] (q's final cast gates the
    next chunk's prefetched scores; v's last pieces are only read by the
    final AV batch).  Each chunk ends with a DVE copy of the raw AV
    accumulator PSUM->SBUF and a contiguous [128, 512] f32 store; the
    last chunk ships columns [0, 384) right after AV(14) so the drain
    only waits on a 64KB tail.
"""

import math

import numpy as np

B, S, E, H = 4, 2048, 512, 8
DK_H = DV_H = 8
NCORES = 8
HPC = H // 2  # heads per core = 4
SCALE = 1.0 / math.sqrt(DK_H)
NQC = S // 512  # q chunks of 512
NTB = S // 128  # t blocks of 128
ECH = E // 128  # e chunks of 128

_cache: dict = {}


def _apply_tile_patch():
    """walrus in this image allows only one sync-wait per Drain; split the
    TileContext tail drain's waits across a chain of drains."""
    import concourse.mybir as mybir
    from concourse import tile
    from concourse.vector_clock import ScopedClock

    if getattr(tile.TileContext._drain_and_barrier, "_split_patch", False):
        return

    def _drain_and_barrier_split(self, tick_clock, wait_clock):
        drain_inst = self.nc.sync.drain()
        wait_clock.add_sem_waits(
            drain_inst.ins, ScopedClock({None: tick_clock.global_clock})
        )
        si = drain_inst.ins.sync_info
        if si is not None and si.on_wait and len(si.on_wait) > 1:
            waits = list(si.on_wait)
            si.on_wait = waits[:1]
            for entry in waits[1:]:
                extra = self.nc.sync.drain()
                extra.ins.sync_info = mybir.SyncInfo(on_wait=[entry], on_update=[])
        self.nc.all_engine_barrier()
        assert self.sems is not None
        popped = self.nc._tile_sem_poison_stack.pop()
        assert popped is self._sem_poison
        self.nc.clear_and_free_semaphores(list(self.sems.allocated().values()))
        self.nc.all_engine_barrier()

    _drain_and_barrier_split._split_patch = True
    tile.TileContext._drain_and_barrier = _drain_and_barrier_split


def _split_multi_waits(nc):
    """walrus in this image allows only one sync-wait per instruction;
    move excess waits onto single-wait NOPs inserted just before."""
    import concourse.mybir as mybir

    for blk in nc.m.functions[0].blocks:
        out = []
        for inst in blk.instructions:
            si = getattr(inst, "sync_info", None)
            if si is not None and si.on_wait and len(si.on_wait) > 1:
                waits = list(si.on_wait)
                for i, entry in enumerate(waits[:-1]):
                    out.append(
                        mybir.InstNoOp(
                            name=f"{inst.name}_w{i}",
                            engine=inst.engine,
                            ins=[],
                            outs=[],
                            bass_nofuse=True,
                            sync_info=mybir.SyncInfo(
                                on_wait=[entry], on_update=[]
                            ),
                        )
                    )
                si.on_wait = waits[-1:]
            out.append(inst)
        blk.instructions = out


def _build():
    import concourse.bass as bassmod
    import concourse.mybir as mybir
    from concourse import tile

    _apply_tile_patch()
    f32 = mybir.dt.float32
    bf16 = mybir.dt.bfloat16
    Exp = mybir.ActivationFunctionType.Exp

    nc = bassmod.Bass()
    # q/k/v host-packed into contiguous [128, 512] tiles, (e, chunk)-major,
    # so every input DMA is one contiguous 128KB burst instead of a
    # 1KB-of-4KB strided read
    qT = nc.declare_dram_parameter("qT", [ECH * NQC * 128, 512], bf16,
                                   isOutput=False)
    kT = nc.declare_dram_parameter("kT", [ECH * NQC * 128, 512], bf16,
                                   isOutput=False)
    vT = nc.declare_dram_parameter("vT", [ECH * NQC * 128, 512], bf16,
                                   isOutput=False)
    # weights host-packed partition-major so their DMAs are contiguous
    wq = nc.declare_dram_parameter("wq", [128, ECH * 128], bf16, isOutput=False)
    wk = nc.declare_dram_parameter("wk", [128, ECH * 128], bf16, isOutput=False)
    wv = nc.declare_dram_parameter("wv", [128, ECH * HPC * 9], bf16,
                                   isOutput=False)
    msk = nc.declare_dram_parameter("msk", [128, 2 * 128], bf16, isOutput=False)
    # output chunk-major: each chunk's [128, 512] store is contiguous
    out = nc.declare_dram_parameter("out", [NQC * 128, 512], f32,
                                    isOutput=True)

    with tile.TileContext(nc) as tc:
        with (
            tc.tile_pool(name="singles", bufs=1) as singles,
            tc.tile_pool(name="loads", bufs=12) as loads,
            tc.tile_pool(name="abuf", bufs=12) as abuf,
            tc.tile_pool(name="outs", bufs=2) as outs,
            tc.tile_pool(name="ps_sc", bufs=3, space="PSUM") as ps_sc,
            tc.tile_pool(name="ps_av", bufs=1, space="PSUM") as ps_av,
            tc.tile_pool(name="ps_misc", bufs=1, space="PSUM") as ps_misc,
        ):
            # ---- resident tensors -------------------------------------
            wq_sb = singles.tile([128, ECH, 128], bf16, tag="wq")
            wk_sb = singles.tile([128, ECH, 128], bf16, tag="wk")
            wv_sb = singles.tile([128, ECH, HPC * 9], bf16, tag="wv")
            msk_sb = singles.tile([128, 2, 128], bf16, tag="msk")
            # startup: K path fully on the Sync DMA queue, Q path (incl. its
            # weight) on GpSimd, so both projection chains run in parallel;
            # wv/msk are deferred until after the q tiles are queued
            nc.gpsimd.dma_start(out=wq_sb, in_=wq.rearrange("p (c m) -> p c m", c=ECH))
            nc.sync.dma_start(out=wk_sb, in_=wk.rearrange("p (c m) -> p c m", c=ECH))

            def late_weights():
                nc.gpsimd.dma_start(
                    out=wv_sb, in_=wv.rearrange("p (c m) -> p c m", c=ECH)
                )
                nc.gpsimd.dma_start(
                    out=msk_sb, in_=msk.rearrange("p (g n) -> p g n", g=2)
                )

            # per-chunk projected tensors (separate tiles so the tile
            # dependency tracker never serializes chunk c's reads against
            # chunk c+2's writes)
            KT_t = [
                singles.tile([128, 512], bf16, tag=f"KT{c}", name=f"KT{c}")
                for c in range(NQC)
            ]
            QT_t = [
                singles.tile([128, 512], bf16, tag=f"QT{c}", name=f"QT{c}")
                for c in range(NQC)
            ]
            V_t = [
                singles.tile([128, 4, HPC, 9], bf16, tag=f"V{c}", name=f"V{c}")
                for c in range(NQC)
            ]

            ones9 = singles.tile([128, 9], bf16, tag="ones9")
            nc.vector.memset(ones9, 1.0)
            for c in range(NQC):
                nc.vector.memset(V_t[c][:, :, :, 0:1], 1.0)

            dmaq = [nc.sync, nc.gpsimd]

            def proj_pieces(c, lo_first=False):
                """Q/K/V projections for chunk c as dicts of emission
                closures (<=0.7us of tensor work each) so pieces fit a
                slot's tensor headroom without starving the exp pipeline.
                With lo_first, the q/k pieces emit only the [0:64] cast
                (heads 0-1); the [64:128] casts are exposed as "q_hi" /
                "k_hi" closures so the first score group can start before
                the second half is cast."""
                cs = slice(c * 512, (c + 1) * 512)
                st = {}

                def dma_in(src, key, n):
                    tiles = []
                    for e in range(ECH):
                        if c == 0 and key in ("k", "q"):
                            q = dmaq[0 if key == "k" else 1]
                        else:
                            q = dmaq[(e + n) % 2]
                        t = loads.tile([128, 512], bf16, tag="ld",
                                       name=f"{key}{c}_{e}")
                        r0 = (c * ECH + e) * 128
                        q.dma_start(out=t, in_=src[r0:r0 + 128, :])
                        tiles.append(t)
                    st[key] = tiles

                def cast_half(key, dst, g):
                    nc.vector.tensor_copy(
                        dst[64 * g:64 * g + 64, :],
                        st[key + "ps"][64 * g:64 * g + 64, :],
                    )

                st["cast"] = cast_half

                def qk_mm(key, w_sb, dst, e, casts=(0, 1)):
                    if e == 0:
                        if c == 0 and key in ("q", "k"):
                            # startup: borrow score-pool buffers (idle
                            # until the first scores) so the K and Q
                            # projection chains run in parallel despite
                            # ps_misc having a single buffer
                            qt0 = ps_sc.tile([128, 2, 512], f32, tag="sc",
                                             name=f"{key}ps0")
                            st[key + "ps"] = qt0[:, 0, :]
                        else:
                            st[key + "ps"] = ps_misc.tile(
                                [128, 512], f32, tag="ps", name=f"{key}ps{c}"
                            )
                    if c == 0 and key in ("q", "k"):
                        # startup: nothing else is in flight to co-stream
                        # with, so split into two column-tile halves that
                        # pair with each other (halves the wall time)
                        for g in range(2):
                            nc.tensor.matmul(
                                st[key + "ps"][64 * g:64 * g + 64, :],
                                w_sb[:, e, 64 * g:64 * g + 64],
                                st[key][e][:, :],
                                start=(e == 0), stop=(e == ECH - 1),
                                tile_position=(0, 64 * g),
                            )
                    else:
                        # steady state: solo full-width matmul — splitting
                        # is wall-neutral for the pair itself but blocks
                        # BOTH PE stream lanes; a solo leaves the second
                        # lane free for score/AV pairs to ride
                        nc.tensor.matmul(
                            st[key + "ps"], w_sb[:, e, :], st[key][e][:, :],
                            start=(e == 0), stop=(e == ECH - 1),
                        )
                    if e == ECH - 1:
                        # split the cast so the first score batch (heads
                        # 0-1, partitions < 64) gates on the first half
                        for g in casts:
                            cast_half(key, dst, g)

                def v_mm(tb, half):
                    if half == 0:
                        st[f"vps{tb}"] = ps_misc.tile(
                            [128, HPC * 9], f32, tag="ps", name=f"vps{c}_{tb}"
                        )
                    for e in (0, 1) if half == 0 else (2, 3):
                        nc.tensor.matmul(
                            st[f"vps{tb}"],
                            st["v"][e][:, tb * 128:(tb + 1) * 128],
                            wv_sb[:, e, :],
                            start=(e == 0), stop=(e == ECH - 1),
                        )
                    if half == 1:
                        dst = V_t[c][:, tb, :, 1:9]
                        src = st[f"vps{tb}"].rearrange(
                            "p (h n) -> p h n", n=9
                        )[:, :, 1:9]
                        nc.vector.tensor_copy(dst, src)

                qkcasts = (0,) if lo_first else (0, 1)
                return {
                    "q": [
                        lambda e=e: (
                            dma_in(qT, "q", 0) if e == 0 else None,
                            qk_mm("q", wq_sb, QT_t[c], e, qkcasts),
                        )
                        for e in range(ECH)
                    ],
                    "k": [
                        lambda e=e: (
                            dma_in(kT, "k", 1) if e == 0 else None,
                            qk_mm("k", wk_sb, KT_t[c], e, qkcasts),
                        )
                        for e in range(ECH)
                    ],
                    "q_hi": lambda: cast_half("q", QT_t[c], 1),
                    "k_hi": lambda: cast_half("k", KT_t[c], 1),
                    "v": [
                        lambda tb=tb, half=half: (
                            dma_in(vT, "v", 0)
                            if (tb == 0 and half == 0) else None,
                            v_mm(tb, half),
                        )
                        for tb in range(4)
                        for half in range(2)
                    ],
                }

            # ---- attention, software-pipelined ------------------------
            def emit_sa(c, tb, gs=(0, 1), ats=None):
                """Scores + mask + exp for (chunk c, t-block tb), head
                groups `gs` (pass ats back in to finish a partial block)."""
                d = 128 * tb - 512 * c  # diagonal offset within chunk
                vstart = max(d, 0)
                if ats is None:
                    ats = [None, None]
                scg = {}
                # emit ALL score matmuls before the activations: the four
                # row-disjoint tiles can co-stream wider than pairwise when
                # they queue together
                for g in gs:
                    scg[g] = ps_sc.tile([128, 2, 512], f32, tag="sc",
                                        name=f"sc{c}_{tb}_{g}")
                    ats[g] = abuf.tile([128, 2, 512], bf16, tag="a",
                                       name=f"a{c}_{tb}_{g}")
                    for j in range(2):
                        h = 2 * g + j
                        nc.tensor.matmul(
                            scg[g][:, j, vstart:512],
                            KT_t[tb // 4][32 * h:32 * h + 8,
                                          (tb % 4) * 128:(tb % 4 + 1) * 128],
                            QT_t[c][32 * h:32 * h + 8, vstart:512],
                            start=True, stop=True,
                            tile_position=(32 * h, 0),
                        )
                for g in gs:
                    nc.scalar.activation(
                        ats[g][:, :, vstart:512], scg[g][:, :, vstart:512],
                        Exp, scale=SCALE,
                    )
                    if d >= 0:
                        # zero the upper triangle AFTER exp (bf16 SBUF mul
                        # runs in the DVE 2x mode and keeps the scores->exp
                        # chain free of DVE hops)
                        nc.vector.tensor_mul(
                            ats[g][:, :, d:d + 128],
                            ats[g][:, :, d:d + 128],
                            msk_sb,
                        )
                return ats, vstart

            # prefix: chunk 0's K/Q projections inline, first scores/exp,
            # then chunk 0's first V block and chunk 1's Q/K — all during
            # the act-idle startup window
            sa_pre = {}  # (c, tb) -> (ats, vstart) emitted ahead of its chunk
            p0 = proj_pieces(0, lo_first=True)
            pp = {c: proj_pieces(c) for c in range(1, NQC)}
            for f in p0["k"]:
                f()
            for f in p0["q"]:
                f()
            late_weights()
            # preload the exp activation table (1.3us) on the scalar queue
            # AFTER its startup DMA issues, still ahead of the first scores
            warm = abuf.tile([128, 2], bf16, tag="warm")
            nc.scalar.activation(warm, ones9[:, 0:2], Exp, scale=1.0)
            # first score group fires on the lo casts alone; the hi casts
            # run on DVE while the act engine chews on group 0
            ats00, vs00 = emit_sa(0, 0, gs=(0,))
            p0["k_hi"]()
            p0["q_hi"]()
            emit_sa(0, 0, gs=(1,), ats=ats00)
            sa_pre[(0, 0)] = (ats00, vs00)
            p0["v"][0]()
            p0["v"][1]()
            for f in pp[1]["q"]:
                f()
            for f in pp[1]["k"]:
                f()

            work = []  # FIFO of deferred emission closures
            avs = {}
            for c in range(NQC):
                # no memset: every AV chain starts with start=True, and the
                # host only reads rows 32h..32h+9, which are always written
                av = avs[c] = ps_av.tile([128, 512], f32, tag="av",
                                         name=f"av{c}")
                ntb = 4 * (c + 1)

                if c == 0:
                    work.extend(p0["v"][2:8])  # V1-3(0)
                else:
                    # q(c+1) BEFORE v(c): q's final cast gates the next
                    # chunk's prefetched scores at the boundary, while
                    # v(c)'s last pieces are only read by this chunk's
                    # final AV batch
                    if c >= 2:
                        work.extend(pp[c]["k"])
                    if c + 1 < NQC:
                        work.extend(pp[c + 1]["q"])
                    else:
                        work.extend([None, None, None])
                    work.extend(pp[c]["v"])

                pend = None  # (tb, ats, vstart) waiting for its AV matmuls
                for tb in range(ntb):
                    if (c, tb) in sa_pre:
                        ats, vstart = sa_pre.pop((c, tb))
                    else:
                        ats, vstart = emit_sa(c, tb)
                    # AV for the PREVIOUS tb — keeps exp ahead of the PE
                    if pend is not None:
                        ptb, pats, pvs = pend
                        for h in range(HPC):
                            g, j = divmod(h, 2)
                            nc.tensor.matmul(
                                av[32 * h:32 * h + 9, pvs:512],
                                V_t[ptb // 4][:, ptb % 4, h, :],
                                pats[g][:, j, pvs:512],
                                start=(ptb == 0), stop=False,
                                tile_position=(0, 32 * h),
                            )
                    pend = (tb, ats, vstart)
                    # drain the FIFO evenly across this chunk's slots,
                    # holding back the first 2 slots (c>0) so the boundary
                    # scores are never stuck behind pieces
                    hold = 0 if c == 0 else 2
                    if tb >= hold:
                        npop = -(-len(work) // (ntb - tb))  # ceil
                        for _ in range(npop):
                            piece = work.pop(0)
                            if piece is not None:
                                piece()
                # prefetch the next chunk's first TWO score/exp groups
                # ahead of the final AV batch so the act pipeline never
                # drains at the boundary (the second group would otherwise
                # queue behind the final-AV matmuls)
                if c + 1 < NQC:
                    sa_pre[(c + 1, 0)] = emit_sa(c + 1, 0)
                    sa_pre[(c + 1, 1)] = emit_sa(c + 1, 1)
                av_sb = outs.tile([128, 512], f32, tag="avsb",
                                  name=f"avsb{c}")
                if c == NQC - 1:
                    # tail: columns [0, pvs) are final once AV(ntb-2) is in
                    # (the last block only touches [pvs, 512)) — ship them
                    # now so the drain doesn't wait on the big transfer
                    lo = pend[2]  # = vstart of the final t-block
                    nc.vector.tensor_copy(av_sb[:, 0:lo], av[:, 0:lo])
                    dmaq[c % 2].dma_start(
                        out=out[c * 128:(c + 1) * 128, 0:lo],
                        in_=av_sb[:, 0:lo]
                    )
                else:
                    lo = 0
                ptb, pats, pvs = pend
                for h in range(HPC):
                    g, j = divmod(h, 2)
                    nc.tensor.matmul(
                        av[32 * h:32 * h + 9, pvs:512],
                        V_t[ptb // 4][:, ptb % 4, h, :],
                        pats[g][:, j, pvs:512],
                        start=(ptb == 0), stop=True,
                        tile_position=(0, 32 * h),
                    )
                # ship the raw accumulator (numerators + denominators):
                # PSUM -> SBUF copy on DVE, then DMA out.  Emitted here
                # (before the next chunk's av memset in DVE program order)
                # so the bufs=2 rotation can never deadlock.
                nc.vector.tensor_copy(av_sb[:, lo:512], av[:, lo:512])
                dmaq[c % 2].dma_start(
                    out=out[c * 128:(c + 1) * 128, lo:512],
                    in_=av_sb[:, lo:512]
                )
    _split_multi_waits(nc)
    return nc


def _prep_inputs(query, key, value, Wq, Wk, Wv):
    """Build the 8 per-core input maps (host-side sharding/layout)."""
    import ml_dtypes

    bf16 = np.dtype(ml_dtypes.bfloat16)

    def packT(x):  # [S, E] -> tile-packed [(chunk e p), 512] bf16
        xt = np.ascontiguousarray(x.T)  # [E, S]
        return (
            xt.reshape(ECH, 128, NQC, 512).transpose(2, 0, 1, 3)
            .astype(bf16).reshape(ECH * NQC * 128, 512)
        )

    qTs = [packT(query[b]) for b in range(B)]
    kTs = [packT(key[b]) for b in range(B)]
    vTs = [packT(value[b]) for b in range(B)]

    mask = np.where(
        np.arange(128)[:, None] <= np.arange(128)[None, :], 1.0, 0.0
    ).astype(np.float32)
    msk2 = np.ascontiguousarray(np.tile(mask, (1, 2))).astype(bf16)

    in_maps = []
    for core in range(NCORES):
        b, hh = divmod(core, 2)
        wq_p = np.zeros((E, 128), np.float32)
        wk_p = np.zeros((E, 128), np.float32)
        wv_p = np.zeros((E, HPC * 9), np.float32)
        for h in range(HPC):
            g = 4 * hh + h
            wq_p[:, 32 * h:32 * h + 8] = Wq[g]
            wk_p[:, 32 * h:32 * h + 8] = Wk[g]
            wv_p[:, 9 * h + 1:9 * h + 9] = Wv[g]
        def pack(w):  # [E, M] -> [128, ECH*M] partition-major
            m = w.shape[1]
            return np.ascontiguousarray(
                w.reshape(ECH, 128, m).transpose(1, 0, 2).reshape(128, ECH * m)
            )

        in_maps.append(
            {
                "qT": qTs[b], "kT": kTs[b], "vT": vTs[b],
                "wq": pack(wq_p).astype(bf16), "wk": pack(wk_p).astype(bf16),
                "wv": pack(wv_p).astype(bf16),
                "msk": msk2,
            }
        )
    return in_maps


def _reference_numpy(query, key, value, padding_mask, decoder_mask,
                     Wq, Wk, Wv, Wo, bo):
    """Fallback (non-default masks): plain numpy replica of the reference."""
    q = np.einsum("bse,hed->bhsd", query, Wq)
    k = np.einsum("bse,hed->bhsd", key, Wk)
    v = np.einsum("bse,hed->bhsd", value, Wv)
    s = np.einsum("bhsd,bhtd->bhst", q, k)
    if decoder_mask:
        tril = np.tril(s)
        s = np.where(tril == 0.0, -np.inf, s)
    s = np.where(padding_mask[:, None, :, :], s, -np.inf)
    s = s / np.sqrt(np.float32(DK_H))
    m = np.max(s, axis=-1, keepdims=True)
    e = np.exp(s - m)
    a = e / np.sum(e, axis=-1, keepdims=True)
    o = np.einsum("bhst,bhtd->bhsd", a, v)
    o = o.transpose(0, 2, 1, 3).reshape(o.shape[0], o.shape[2], H * DV_H)
    return (o @ Wo + bo).astype(np.float32)


def kernel(query, key, value, padding_mask, decoder_mask, Wq, Wk, Wv, Wo, bo,
           **run_kwargs):
    query = np.asarray(query, np.float32)
    key = np.asarray(key, np.float32)
    value = np.asarray(value, np.float32)
    Wq = np.asarray(Wq, np.float32)
    Wk = np.asarray(Wk, np.float32)
    Wv = np.asarray(Wv, np.float32)
    Wo = np.asarray(Wo, np.float32)
    bo = np.asarray(bo, np.float32)
    pm = np.asarray(padding_mask)
    dm = int(np.asarray(decoder_mask))

    if not bool(pm.all()) or not dm:
        return _reference_numpy(
            query, key, value, pm.astype(bool), dm, Wq, Wk, Wv, Wo, bo
        )

    from concourse.bass_utils import run_bass_kernel_spmd

    if "nc" not in _cache:
        _cache["nc"] = _build()
    nc = _cache["nc"]

    in_maps = _prep_inputs(query, key, value, Wq, Wk, Wv)
    res = run_bass_kernel_spmd(nc, in_maps, list(range(NCORES)), **run_kwargs)

    # host finish: divide numerators by denominators, concatenate heads,
    # apply the output projection + bias (all f32)
    outp = np.empty((B, S, E), np.float32)
    ov = np.empty((S, H * DV_H), np.float32)
    for b in range(B):
        for hh in range(2):
            # chunk-major [NQC*128, 512] -> [128, S]
            r = res.results[2 * b + hh]["out"]
            r = r.reshape(NQC, 128, 512).transpose(1, 0, 2).reshape(128, S)
            for h in range(HPC):
                g = 4 * hh + h
                den = r[32 * h]
                num = r[32 * h + 1:32 * h + 9]
                ov[:, 8 * g:8 * g + 8] = (num / den).T
        outp[b] = ov @ Wo + bo
    if run_kwargs:
        kernel.last_result = res
    return outp


# revision 57
# speedup vs baseline: 1.1211x; 1.0429x over previous
"""Bass/Trainium2 kernel for nn_MultiHeadAttention (B=4, S=2048, E=512, H=8, dk=dv=8).

Sharding: 8 cores = (batch b, head-half hh).  Core 2b+hh computes causal
attention for batch b over heads [4hh, 4hh+4) for all 2048 queries and
returns the UNNORMALIZED attention accumulator per chunk: rows 32h hold
the softmax denominator (ones column in V), rows 32h+1..32h+9 the
numerator (exp(s) @ V_h).  The host divides, concatenates heads, and
applies the output projection + bias in f32 (cheap: 0.5 GFLOP total).

Device layout notes (v4 — host-finish, deep score buffering, packed DMA):
  - Host feeds query/key/value as host-packed CONTIGUOUS [128, 512] bf16
    tiles, (chunk, e)-major, so every input DMA is a single 128KB burst
    (the [E, S] view's 1KB-of-4KB strided reads throttled startup).
  - Projection weights are host-packed bf16 "spread" layouts: Q/K outputs
    land at partitions {32h+d}; V outputs at columns {9h+d} with a ones
    column per head at 9h+0 accumulating the softmax denominator.
  - The PE power throttle caps each matmul stream at ~0.78 cols/ns, but
    two tile-disjoint matmuls co-stream at full rate each (measured; >2
    rarely co-streams, and matmul PSUM destinations must be 2KB-bank-
    aligned, which forbids packing 4 quad-issued heads into fewer banks).
    Scores pair via row-tiling (tile_position=(32h, 0), K=8); A@V pairs
    via col-tiling (tile_position=(0, 32h), M=9) and sits at the PE's
    ~205 G elem/s element cap.  Q/K projections run as full-width solos
    in steady state (splitting them only burns the second stream lane);
    chunk 0's run as co-streaming column-tile halves since nothing else
    is in flight at startup.
  - Scores are computed transposed ([t, q]); exp outputs bf16 tiles that
    feed the A@V matmul as the moving operand (V stationary).  The causal
    mask is applied AFTER exp as a bf16 0/1 multiply (keeps the
    scores->exp chain free of DVE hops).
  - PSUM: score tiles [128, 2, 512] f32 x3 bufs (6 banks) so a score
    pair is released three activations back; av accumulator x1 (no
    memset — AV chains start with start=True and the host ignores
    unwritten rows); projection psum x1 (chunk-0's K/Q borrow idle
    score-pool buffers to keep their chains parallel at startup).
  - Emission is software-pipelined: scores(tb+1) are emitted before
    AV(tb); projections are diced into <=0.7us closures drained via a
    work FIFO ordered [k(c), q(c+1), v(c)] (q's final cast gates the
    next chunk's prefetched scores; v's last pieces are only read by the
    final AV batch).  Each chunk ends with a DVE copy of the raw AV
    accumulator PSUM->SBUF and a contiguous [128, 512] f32 store; the
    last chunk ships columns [0, 384) right after AV(14) so the drain
    only waits on a 64KB tail.
"""

import math

import numpy as np

B, S, E, H = 4, 2048, 512, 8
DK_H = DV_H = 8
NCORES = 8
HPC = H // 2  # heads per core = 4
SCALE = 1.0 / math.sqrt(DK_H)
NQC = S // 512  # q chunks of 512
NTB = S // 128  # t blocks of 128
ECH = E // 128  # e chunks of 128

_cache: dict = {}


def _apply_tile_patch():
    """walrus in this image allows only one sync-wait per Drain; split the
    TileContext tail drain's waits across a chain of drains."""
    import concourse.mybir as mybir
    from concourse import tile
    from concourse.vector_clock import ScopedClock

    if getattr(tile.TileContext._drain_and_barrier, "_split_patch", False):
        return

    def _drain_and_barrier_split(self, tick_clock, wait_clock):
        drain_inst = self.nc.sync.drain()
        wait_clock.add_sem_waits(
            drain_inst.ins, ScopedClock({None: tick_clock.global_clock})
        )
        si = drain_inst.ins.sync_info
        if si is not None and si.on_wait and len(si.on_wait) > 1:
            waits = list(si.on_wait)
            si.on_wait = waits[:1]
            for entry in waits[1:]:
                extra = self.nc.sync.drain()
                extra.ins.sync_info = mybir.SyncInfo(on_wait=[entry], on_update=[])
        self.nc.all_engine_barrier()
        assert self.sems is not None
        popped = self.nc._tile_sem_poison_stack.pop()
        assert popped is self._sem_poison
        self.nc.clear_and_free_semaphores(list(self.sems.allocated().values()))
        self.nc.all_engine_barrier()

    _drain_and_barrier_split._split_patch = True
    tile.TileContext._drain_and_barrier = _drain_and_barrier_split


def _split_multi_waits(nc):
    """walrus in this image allows only one sync-wait per instruction;
    move excess waits onto single-wait NOPs inserted just before."""
    import concourse.mybir as mybir

    for blk in nc.m.functions[0].blocks:
        out = []
        for inst in blk.instructions:
            si = getattr(inst, "sync_info", None)
            if si is not None and si.on_wait and len(si.on_wait) > 1:
                waits = list(si.on_wait)
                for i, entry in enumerate(waits[:-1]):
                    out.append(
                        mybir.InstNoOp(
                            name=f"{inst.name}_w{i}",
                            engine=inst.engine,
                            ins=[],
                            outs=[],
                            bass_nofuse=True,
                            sync_info=mybir.SyncInfo(
                                on_wait=[entry], on_update=[]
                            ),
                        )
                    )
                si.on_wait = waits[-1:]
            out.append(inst)
        blk.instructions = out


def _build():
    import concourse.bass as bassmod
    import concourse.mybir as mybir
    from concourse import tile

    _apply_tile_patch()
    f32 = mybir.dt.float32
    bf16 = mybir.dt.bfloat16
    Exp = mybir.ActivationFunctionType.Exp

    nc = bassmod.Bass()
    # q/k/v host-packed into contiguous [128, 512] tiles, (e, chunk)-major,
    # so every input DMA is one contiguous 128KB burst instead of a
    # 1KB-of-4KB strided read
    qT = nc.declare_dram_parameter("qT", [ECH * NQC * 128, 512], bf16,
                                   isOutput=False)
    kT = nc.declare_dram_parameter("kT", [ECH * NQC * 128, 512], bf16,
                                   isOutput=False)
    vT = nc.declare_dram_parameter("vT", [ECH * NQC * 128, 512], bf16,
                                   isOutput=False)
    # weights host-packed partition-major so their DMAs are contiguous
    wq = nc.declare_dram_parameter("wq", [128, ECH * 128], bf16, isOutput=False)
    wk = nc.declare_dram_parameter("wk", [128, ECH * 128], bf16, isOutput=False)
    wv = nc.declare_dram_parameter("wv", [128, ECH * HPC * 9], bf16,
                                   isOutput=False)
    msk = nc.declare_dram_parameter("msk", [128, 2 * 128], bf16, isOutput=False)
    # output chunk-major: each chunk's [128, 512] store is contiguous
    out = nc.declare_dram_parameter("out", [NQC * 128, 512], f32,
                                    isOutput=True)

    with tile.TileContext(nc) as tc:
        with (
            tc.tile_pool(name="singles", bufs=1) as singles,
            tc.tile_pool(name="loads", bufs=12) as loads,
            tc.tile_pool(name="abuf", bufs=12) as abuf,
            tc.tile_pool(name="outs", bufs=2) as outs,
            tc.tile_pool(name="ps_sc", bufs=3, space="PSUM") as ps_sc,
            tc.tile_pool(name="ps_av", bufs=1, space="PSUM") as ps_av,
            tc.tile_pool(name="ps_misc", bufs=1, space="PSUM") as ps_misc,
        ):
            # ---- resident tensors -------------------------------------
            wq_sb = singles.tile([128, ECH, 128], bf16, tag="wq")
            wk_sb = singles.tile([128, ECH, 128], bf16, tag="wk")
            wv_sb = singles.tile([128, ECH, HPC * 9], bf16, tag="wv")
            msk_sb = singles.tile([128, 2, 128], bf16, tag="msk")
            # startup: K path fully on the Sync DMA queue, Q path (incl. its
            # weight) on GpSimd, so both projection chains run in parallel;
            # wv/msk are deferred until after the q tiles are queued
            nc.gpsimd.dma_start(out=wq_sb, in_=wq.rearrange("p (c m) -> p c m", c=ECH))
            nc.sync.dma_start(out=wk_sb, in_=wk.rearrange("p (c m) -> p c m", c=ECH))

            def late_weights():
                nc.gpsimd.dma_start(
                    out=wv_sb, in_=wv.rearrange("p (c m) -> p c m", c=ECH)
                )
                nc.gpsimd.dma_start(
                    out=msk_sb, in_=msk.rearrange("p (g n) -> p g n", g=2)
                )

            # per-chunk projected tensors (separate tiles so the tile
            # dependency tracker never serializes chunk c's reads against
            # chunk c+2's writes)
            KT_t = [
                singles.tile([128, 512], bf16, tag=f"KT{c}", name=f"KT{c}")
                for c in range(NQC)
            ]
            QT_t = [
                singles.tile([128, 512], bf16, tag=f"QT{c}", name=f"QT{c}")
                for c in range(NQC)
            ]
            V_t = [
                singles.tile([128, 4, HPC, 9], bf16, tag=f"V{c}", name=f"V{c}")
                for c in range(NQC)
            ]

            ones9 = singles.tile([128, 9], bf16, tag="ones9")
            nc.vector.memset(ones9, 1.0)
            for c in range(NQC):
                nc.vector.memset(V_t[c][:, :, :, 0:1], 1.0)

            dmaq = [nc.sync, nc.gpsimd]

            def proj_pieces(c, lo_first=False):
                """Q/K/V projections for chunk c as dicts of emission
                closures (<=0.7us of tensor work each) so pieces fit a
                slot's tensor headroom without starving the exp pipeline.
                With lo_first, the q/k pieces emit only the [0:64] cast
                (heads 0-1); the [64:128] casts are exposed as "q_hi" /
                "k_hi" closures so the first score group can start before
                the second half is cast."""
                cs = slice(c * 512, (c + 1) * 512)
                st = {}

                def dma_in(src, key, n):
                    tiles = []
                    for e in range(ECH):
                        if c == 0 and key in ("k", "q"):
                            q = dmaq[0 if key == "k" else 1]
                        else:
                            q = dmaq[(e + n) % 2]
                        t = loads.tile([128, 512], bf16, tag="ld",
                                       name=f"{key}{c}_{e}")
                        r0 = (c * ECH + e) * 128
                        q.dma_start(out=t, in_=src[r0:r0 + 128, :])
                        tiles.append(t)
                    st[key] = tiles

                def cast_half(key, dst, g):
                    nc.vector.tensor_copy(
                        dst[64 * g:64 * g + 64, :],
                        st[key + "ps"][64 * g:64 * g + 64, :],
                    )

                st["cast"] = cast_half

                def qk_mm(key, w_sb, dst, e, casts=(0, 1)):
                    if e == 0:
                        if c == 0 and key in ("q", "k"):
                            # startup: borrow score-pool buffers (idle
                            # until the first scores) so the K and Q
                            # projection chains run in parallel despite
                            # ps_misc having a single buffer
                            qt0 = ps_sc.tile([128, 2, 512], f32, tag="sc",
                                             name=f"{key}ps0")
                            st[key + "ps"] = qt0[:, 0, :]
                        else:
                            st[key + "ps"] = ps_misc.tile(
                                [128, 512], f32, tag="ps", name=f"{key}ps{c}"
                            )
                    if c == 0 and key in ("q", "k"):
                        # startup: nothing else is in flight to co-stream
                        # with, so split into two column-tile halves that
                        # pair with each other (halves the wall time)
                        for g in range(2):
                            nc.tensor.matmul(
                                st[key + "ps"][64 * g:64 * g + 64, :],
                                w_sb[:, e, 64 * g:64 * g + 64],
                                st[key][e][:, :],
                                start=(e == 0), stop=(e == ECH - 1),
                                tile_position=(0, 64 * g),
                            )
                    else:
                        # steady state: solo full-width matmul — splitting
                        # is wall-neutral for the pair itself but blocks
                        # BOTH PE stream lanes; a solo leaves the second
                        # lane free for score/AV pairs to ride
                        nc.tensor.matmul(
                            st[key + "ps"], w_sb[:, e, :], st[key][e][:, :],
                            start=(e == 0), stop=(e == ECH - 1),
                        )
                    if e == ECH - 1:
                        # split the cast so the first score batch (heads
                        # 0-1, partitions < 64) gates on the first half
                        for g in casts:
                            cast_half(key, dst, g)

                def v_mm(tb, half):
                    if half == 0:
                        st[f"vps{tb}"] = ps_misc.tile(
                            [128, HPC * 9], f32, tag="ps", name=f"vps{c}_{tb}"
                        )
                    for e in (0, 1) if half == 0 else (2, 3):
                        nc.tensor.matmul(
                            st[f"vps{tb}"],
                            st["v"][e][:, tb * 128:(tb + 1) * 128],
                            wv_sb[:, e, :],
                            start=(e == 0), stop=(e == ECH - 1),
                        )
                    if half == 1:
                        dst = V_t[c][:, tb, :, 1:9]
                        src = st[f"vps{tb}"].rearrange(
                            "p (h n) -> p h n", n=9
                        )[:, :, 1:9]
                        nc.vector.tensor_copy(dst, src)

                qkcasts = (0,) if lo_first else (0, 1)
                return {
                    "q": [
                        lambda e=e: (
                            dma_in(qT, "q", 0) if e == 0 else None,
                            qk_mm("q", wq_sb, QT_t[c], e, qkcasts),
                        )
                        for e in range(ECH)
                    ],
                    "k": [
                        lambda e=e: (
                            dma_in(kT, "k", 1) if e == 0 else None,
                            qk_mm("k", wk_sb, KT_t[c], e, qkcasts),
                        )
                        for e in range(ECH)
                    ],
                    "q_hi": lambda: cast_half("q", QT_t[c], 1),
                    "k_hi": lambda: cast_half("k", KT_t[c], 1),
                    "v": [
                        lambda tb=tb, half=half: (
                            dma_in(vT, "v", 0)
                            if (tb == 0 and half == 0) else None,
                            v_mm(tb, half),
                        )
                        for tb in range(4)
                        for half in range(2)
                    ],
                }

            # ---- attention, software-pipelined ------------------------
            def emit_sa(c, tb, gs=(0, 1), ats=None):
                """Scores + mask + exp for (chunk c, t-block tb), head
                groups `gs` (pass ats back in to finish a partial block)."""
                d = 128 * tb - 512 * c  # diagonal offset within chunk
                vstart = max(d, 0)
                if ats is None:
                    ats = [None, None]
                scg = {}
                # emit ALL score matmuls before the activations: the four
                # row-disjoint tiles can co-stream wider than pairwise when
                # they queue together
                for g in gs:
                    scg[g] = ps_sc.tile([128, 2, 512], f32, tag="sc",
                                        name=f"sc{c}_{tb}_{g}")
                    ats[g] = abuf.tile([128, 2, 512], bf16, tag="a",
                                       name=f"a{c}_{tb}_{g}")
                    for j in range(2):
                        h = 2 * g + j
                        nc.tensor.matmul(
                            scg[g][:, j, vstart:512],
                            KT_t[tb // 4][32 * h:32 * h + 8,
                                          (tb % 4) * 128:(tb % 4 + 1) * 128],
                            QT_t[c][32 * h:32 * h + 8, vstart:512],
                            start=True, stop=True,
                            tile_position=(32 * h, 0),
                        )
                for g in gs:
                    nc.scalar.activation(
                        ats[g][:, :, vstart:512], scg[g][:, :, vstart:512],
                        Exp, scale=SCALE,
                    )
                    if d >= 0:
                        # zero the upper triangle AFTER exp (bf16 SBUF mul
                        # runs in the DVE 2x mode and keeps the scores->exp
                        # chain free of DVE hops)
                        nc.vector.tensor_mul(
                            ats[g][:, :, d:d + 128],
                            ats[g][:, :, d:d + 128],
                            msk_sb,
                        )
                return ats, vstart

            # prefix: chunk 0's K/Q projections inline, first scores/exp,
            # then chunk 0's first V block and chunk 1's Q/K — all during
            # the act-idle startup window
            sa_pre = {}  # (c, tb) -> (ats, vstart) emitted ahead of its chunk
            p0 = proj_pieces(0, lo_first=True)
            pp = {c: proj_pieces(c) for c in range(1, NQC)}
            for f in p0["k"]:
                f()
            for f in p0["q"]:
                f()
            late_weights()
            # preload the exp activation table (1.3us) on the scalar queue
            # AFTER its startup DMA issues, still ahead of the first scores
            warm = abuf.tile([128, 2], bf16, tag="warm")
            nc.scalar.activation(warm, ones9[:, 0:2], Exp, scale=1.0)
            # first score group fires on the lo casts alone; the hi casts
            # run on DVE while the act engine chews on group 0
            ats00, vs00 = emit_sa(0, 0, gs=(0,))
            p0["k_hi"]()
            p0["q_hi"]()
            emit_sa(0, 0, gs=(1,), ats=ats00)
            sa_pre[(0, 0)] = (ats00, vs00)
            p0["v"][0]()
            p0["v"][1]()
            for f in pp[1]["q"]:
                f()
            for f in pp[1]["k"]:
                f()

            work = []  # FIFO of deferred emission closures
            avs = {}
            for c in range(NQC):
                # no memset: every AV chain starts with start=True, and the
                # host only reads rows 32h..32h+9, which are always written
                av = avs[c] = ps_av.tile([128, 512], f32, tag="av",
                                         name=f"av{c}")
                ntb = 4 * (c + 1)

                if c == 0:
                    work.extend(p0["v"][2:8])  # V1-3(0)
                else:
                    # q(c+1) BEFORE v(c): q's final cast gates the next
                    # chunk's prefetched scores at the boundary, while
                    # v(c)'s last pieces are only read by this chunk's
                    # final AV batch
                    if c >= 2:
                        work.extend(pp[c]["k"])
                    if c + 1 < NQC:
                        work.extend(pp[c + 1]["q"])
                    else:
                        work.extend([None, None, None])
                    work.extend(pp[c]["v"])

                pend = None  # (tb, ats, vstart) waiting for its AV matmuls
                for tb in range(ntb):
                    if (c, tb) in sa_pre:
                        ats, vstart = sa_pre.pop((c, tb))
                    else:
                        ats, vstart = emit_sa(c, tb)
                    # AV for the PREVIOUS tb — keeps exp ahead of the PE
                    if pend is not None:
                        ptb, pats, pvs = pend
                        for h in range(HPC):
                            g, j = divmod(h, 2)
                            nc.tensor.matmul(
                                av[32 * h:32 * h + 9, pvs:512],
                                V_t[ptb // 4][:, ptb % 4, h, :],
                                pats[g][:, j, pvs:512],
                                start=(ptb == 0), stop=False,
                                tile_position=(0, 32 * h),
                            )
                    pend = (tb, ats, vstart)
                    # drain the FIFO evenly across this chunk's slots,
                    # holding back the first 3 slots (c>0) so the boundary
                    # scores are never stuck behind pieces
                    hold = 0 if c == 0 else 3
                    if tb >= hold:
                        npop = -(-len(work) // (ntb - tb))  # ceil
                        for _ in range(npop):
                            piece = work.pop(0)
                            if piece is not None:
                                piece()
                # prefetch the next chunk's first TWO score/exp groups
                # ahead of the final AV batch so the act pipeline never
                # drains at the boundary (the second group would otherwise
                # queue behind the final-AV matmuls)
                if c + 1 < NQC:
                    sa_pre[(c + 1, 0)] = emit_sa(c + 1, 0)
                    sa_pre[(c + 1, 1)] = emit_sa(c + 1, 1)
                av_sb = outs.tile([128, 512], f32, tag="avsb",
                                  name=f"avsb{c}")
                if c == NQC - 1:
                    # tail: columns [0, pvs) are final once AV(ntb-2) is in
                    # (the last block only touches [pvs, 512)) — ship them
                    # now so the drain doesn't wait on the big transfer
                    lo = pend[2]  # = vstart of the final t-block
                    nc.vector.tensor_copy(av_sb[:, 0:lo], av[:, 0:lo])
                    nc.sync.dma_start(
                        out=out[c * 128:(c + 1) * 128, 0:lo],
                        in_=av_sb[:, 0:lo]
                    )
                else:
                    lo = 0
                ptb, pats, pvs = pend
                for h in range(HPC):
                    g, j = divmod(h, 2)
                    nc.tensor.matmul(
                        av[32 * h:32 * h + 9, pvs:512],
                        V_t[ptb // 4][:, ptb % 4, h, :],
                        pats[g][:, j, pvs:512],
                        start=(ptb == 0), stop=True,
                        tile_position=(0, 32 * h),
                    )
                # ship the raw accumulator (numerators + denominators):
                # PSUM -> SBUF copy on DVE, then DMA out.  Emitted here
                # (before the next chunk's av memset in DVE program order)
                # so the bufs=2 rotation can never deadlock.
                nc.vector.tensor_copy(av_sb[:, lo:512], av[:, lo:512])
                # last chunk's store goes on the Sync queue: the exit
                # drain sequence ends on gpsimd, which then doesn't wait
                # on the final output transfer
                (nc.sync if c == NQC - 1 else dmaq[c % 2]).dma_start(
                    out=out[c * 128:(c + 1) * 128, lo:512],
                    in_=av_sb[:, lo:512]
                )
    _split_multi_waits(nc)
    return nc


def _prep_inputs(query, key, value, Wq, Wk, Wv):
    """Build the 8 per-core input maps (host-side sharding/layout)."""
    import ml_dtypes

    bf16 = np.dtype(ml_dtypes.bfloat16)

    def packT(x):  # [S, E] -> tile-packed [(chunk e p), 512] bf16
        xt = np.ascontiguousarray(x.T)  # [E, S]
        return (
            xt.reshape(ECH, 128, NQC, 512).transpose(2, 0, 1, 3)
            .astype(bf16).reshape(ECH * NQC * 128, 512)
        )

    qTs = [packT(query[b]) for b in range(B)]
    kTs = [packT(key[b]) for b in range(B)]
    vTs = [packT(value[b]) for b in range(B)]

    mask = np.where(
        np.arange(128)[:, None] <= np.arange(128)[None, :], 1.0, 0.0
    ).astype(np.float32)
    msk2 = np.ascontiguousarray(np.tile(mask, (1, 2))).astype(bf16)

    in_maps = []
    for core in range(NCORES):
        b, hh = divmod(core, 2)
        wq_p = np.zeros((E, 128), np.float32)
        wk_p = np.zeros((E, 128), np.float32)
        wv_p = np.zeros((E, HPC * 9), np.float32)
        for h in range(HPC):
            g = 4 * hh + h
            wq_p[:, 32 * h:32 * h + 8] = Wq[g]
            wk_p[:, 32 * h:32 * h + 8] = Wk[g]
            wv_p[:, 9 * h + 1:9 * h + 9] = Wv[g]
        def pack(w):  # [E, M] -> [128, ECH*M] partition-major
            m = w.shape[1]
            return np.ascontiguousarray(
                w.reshape(ECH, 128, m).transpose(1, 0, 2).reshape(128, ECH * m)
            )

        in_maps.append(
            {
                "qT": qTs[b], "kT": kTs[b], "vT": vTs[b],
                "wq": pack(wq_p).astype(bf16), "wk": pack(wk_p).astype(bf16),
                "wv": pack(wv_p).astype(bf16),
                "msk": msk2,
            }
        )
    return in_maps


def _reference_numpy(query, key, value, padding_mask, decoder_mask,
                     Wq, Wk, Wv, Wo, bo):
    """Fallback (non-default masks): plain numpy replica of the reference."""
    q = np.einsum("bse,hed->bhsd", query, Wq)
    k = np.einsum("bse,hed->bhsd", key, Wk)
    v = np.einsum("bse,hed->bhsd", value, Wv)
    s = np.einsum("bhsd,bhtd->bhst", q, k)
    if decoder_mask:
        tril = np.tril(s)
        s = np.where(tril == 0.0, -np.inf, s)
    s = np.where(padding_mask[:, None, :, :], s, -np.inf)
    s = s / np.sqrt(np.float32(DK_H))
    m = np.max(s, axis=-1, keepdims=True)
    e = np.exp(s - m)
    a = e / np.sum(e, axis=-1, keepdims=True)
    o = np.einsum("bhst,bhtd->bhsd", a, v)
    o = o.transpose(0, 2, 1, 3).reshape(o.shape[0], o.shape[2], H * DV_H)
    return (o @ Wo + bo).astype(np.float32)


def kernel(query, key, value, padding_mask, decoder_mask, Wq, Wk, Wv, Wo, bo,
           **run_kwargs):
    query = np.asarray(query, np.float32)
    key = np.asarray(key, np.float32)
    value = np.asarray(value, np.float32)
    Wq = np.asarray(Wq, np.float32)
    Wk = np.asarray(Wk, np.float32)
    Wv = np.asarray(Wv, np.float32)
    Wo = np.asarray(Wo, np.float32)
    bo = np.asarray(bo, np.float32)
    pm = np.asarray(padding_mask)
    dm = int(np.asarray(decoder_mask))

    if not bool(pm.all()) or not dm:
        return _reference_numpy(
            query, key, value, pm.astype(bool), dm, Wq, Wk, Wv, Wo, bo
        )

    from concourse.bass_utils import run_bass_kernel_spmd

    if "nc" not in _cache:
        _cache["nc"] = _build()
    nc = _cache["nc"]

    in_maps = _prep_inputs(query, key, value, Wq, Wk, Wv)
    res = run_bass_kernel_spmd(nc, in_maps, list(range(NCORES)), **run_kwargs)

    # host finish: divide numerators by denominators, concatenate heads,
    # apply the output projection + bias (all f32)
    outp = np.empty((B, S, E), np.float32)
    ov = np.empty((S, H * DV_H), np.float32)
    for b in range(B):
        for hh in range(2):
            # chunk-major [NQC*128, 512] -> [128, S]
            r = res.results[2 * b + hh]["out"]
            r = r.reshape(NQC, 128, 512).transpose(1, 0, 2).reshape(128, S)
            for h in range(HPC):
                g = 4 * hh + h
                den = r[32 * h]
                num = r[32 * h + 1:32 * h + 9]
                ov[:, 8 * g:8 * g + 8] = (num / den).T
        outp[b] = ov @ Wo + bo
    if run_kwargs:
        kernel.last_result = res
    return outp


# revision 64
# speedup vs baseline: 1.1212x; 1.0001x over previous
"""Bass/Trainium2 kernel for nn_MultiHeadAttention (B=4, S=2048, E=512, H=8, dk=dv=8).

Sharding: 8 cores = (batch b, head-half hh).  Core 2b+hh computes causal
attention for batch b over heads [4hh, 4hh+4) for all 2048 queries and
returns the UNNORMALIZED attention accumulator per chunk: rows 32h hold
the softmax denominator (ones column in V), rows 32h+1..32h+9 the
numerator (exp(s) @ V_h).  The host divides, concatenates heads, and
applies the output projection + bias in f32 (cheap: 0.5 GFLOP total).

Device layout notes (v4 — host-finish, deep score buffering, packed DMA):
  - Host feeds query/key/value as host-packed CONTIGUOUS [128, 512] bf16
    tiles, (chunk, e)-major, so every input DMA is a single 128KB burst
    (the [E, S] view's 1KB-of-4KB strided reads throttled startup).
  - Projection weights are host-packed bf16 "spread" layouts: Q/K outputs
    land at partitions {32h+d}; V outputs at columns {9h+d} with a ones
    column per head at 9h+0 accumulating the softmax denominator.
  - The PE power throttle caps each matmul stream at ~0.78 cols/ns, but
    two tile-disjoint matmuls co-stream at full rate each (measured; >2
    rarely co-streams, and matmul PSUM destinations must be 2KB-bank-
    aligned, which forbids packing 4 quad-issued heads into fewer banks).
    Scores pair via row-tiling (tile_position=(32h, 0), K=8); A@V pairs
    via col-tiling (tile_position=(0, 32h), M=9) and sits at the PE's
    ~205 G elem/s element cap.  Q/K projections run as full-width solos
    in steady state (splitting them only burns the second stream lane);
    chunk 0's run as co-streaming column-tile halves since nothing else
    is in flight at startup.
  - Scores are computed transposed ([t, q]); exp outputs bf16 tiles that
    feed the A@V matmul as the moving operand (V stationary).  The causal
    mask is applied AFTER exp as a bf16 0/1 multiply (keeps the
    scores->exp chain free of DVE hops).
  - PSUM: score tiles [128, 2, 512] f32 x3 bufs (6 banks) so a score
    pair is released three activations back; av accumulator x1 (no
    memset — AV chains start with start=True and the host ignores
    unwritten rows); projection psum x1 (chunk-0's K/Q borrow idle
    score-pool buffers to keep their chains parallel at startup).
  - Emission is software-pipelined: scores(tb+1) are emitted before
    AV(tb); projections are diced into <=0.7us closures drained via a
    work FIFO ordered [k(c), q(c+1), v(c)] (q's final cast gates the
    next chunk's prefetched scores; v's last pieces are only read by the
    final AV batch).  Each chunk ends with a DVE copy of the raw AV
    accumulator PSUM->SBUF and a contiguous [128, 512] f32 store; the
    last chunk ships columns [0, 384) right after AV(14) so the drain
    only waits on a 64KB tail.
"""

import math

import numpy as np

B, S, E, H = 4, 2048, 512, 8
DK_H = DV_H = 8
NCORES = 8
HPC = H // 2  # heads per core = 4
SCALE = 1.0 / math.sqrt(DK_H)
NQC = S // 512  # q chunks of 512
NTB = S // 128  # t blocks of 128
ECH = E // 128  # e chunks of 128

_cache: dict = {}


def _apply_tile_patch():
    """walrus in this image allows only one sync-wait per Drain; split the
    TileContext tail drain's waits across a chain of drains."""
    import concourse.mybir as mybir
    from concourse import tile
    from concourse.vector_clock import ScopedClock

    if getattr(tile.TileContext._drain_and_barrier, "_split_patch", False):
        return

    def _drain_and_barrier_split(self, tick_clock, wait_clock):
        drain_inst = self.nc.sync.drain()
        wait_clock.add_sem_waits(
            drain_inst.ins, ScopedClock({None: tick_clock.global_clock})
        )
        si = drain_inst.ins.sync_info
        if si is not None and si.on_wait and len(si.on_wait) > 1:
            waits = list(si.on_wait)
            si.on_wait = waits[:1]
            for entry in waits[1:]:
                extra = self.nc.sync.drain()
                extra.ins.sync_info = mybir.SyncInfo(on_wait=[entry], on_update=[])
        self.nc.all_engine_barrier()
        assert self.sems is not None
        popped = self.nc._tile_sem_poison_stack.pop()
        assert popped is self._sem_poison
        self.nc.clear_and_free_semaphores(list(self.sems.allocated().values()))
        self.nc.all_engine_barrier()

    _drain_and_barrier_split._split_patch = True
    tile.TileContext._drain_and_barrier = _drain_and_barrier_split


def _split_multi_waits(nc):
    """walrus in this image allows only one sync-wait per instruction;
    move excess waits onto single-wait NOPs inserted just before."""
    import concourse.mybir as mybir

    for blk in nc.m.functions[0].blocks:
        out = []
        for inst in blk.instructions:
            si = getattr(inst, "sync_info", None)
            if si is not None and si.on_wait and len(si.on_wait) > 1:
                waits = list(si.on_wait)
                for i, entry in enumerate(waits[:-1]):
                    out.append(
                        mybir.InstNoOp(
                            name=f"{inst.name}_w{i}",
                            engine=inst.engine,
                            ins=[],
                            outs=[],
                            bass_nofuse=True,
                            sync_info=mybir.SyncInfo(
                                on_wait=[entry], on_update=[]
                            ),
                        )
                    )
                si.on_wait = waits[-1:]
            out.append(inst)
        blk.instructions = out


def _build():
    import concourse.bass as bassmod
    import concourse.mybir as mybir
    from concourse import tile

    _apply_tile_patch()
    f32 = mybir.dt.float32
    bf16 = mybir.dt.bfloat16
    Exp = mybir.ActivationFunctionType.Exp

    nc = bassmod.Bass()
    # q/k/v host-packed into contiguous [128, 512] tiles, (e, chunk)-major,
    # so every input DMA is one contiguous 128KB burst instead of a
    # 1KB-of-4KB strided read
    qT = nc.declare_dram_parameter("qT", [ECH * NQC * 128, 512], bf16,
                                   isOutput=False)
    kT = nc.declare_dram_parameter("kT", [ECH * NQC * 128, 512], bf16,
                                   isOutput=False)
    vT = nc.declare_dram_parameter("vT", [ECH * NQC * 128, 512], bf16,
                                   isOutput=False)
    # weights host-packed partition-major so their DMAs are contiguous
    wq = nc.declare_dram_parameter("wq", [128, ECH * 128], bf16, isOutput=False)
    wk = nc.declare_dram_parameter("wk", [128, ECH * 128], bf16, isOutput=False)
    wv = nc.declare_dram_parameter("wv", [128, ECH * HPC * 9], bf16,
                                   isOutput=False)
    msk = nc.declare_dram_parameter("msk", [128, 2 * 128], bf16, isOutput=False)
    # output chunk-major: each chunk's [128, 512] store is contiguous
    out = nc.declare_dram_parameter("out", [NQC * 128, 512], f32,
                                    isOutput=True)

    with tile.TileContext(nc) as tc:
        with (
            tc.tile_pool(name="singles", bufs=1) as singles,
            tc.tile_pool(name="loads", bufs=12) as loads,
            tc.tile_pool(name="abuf", bufs=12) as abuf,
            tc.tile_pool(name="outs", bufs=2) as outs,
            tc.tile_pool(name="ps_sc", bufs=3, space="PSUM") as ps_sc,
            tc.tile_pool(name="ps_av", bufs=1, space="PSUM") as ps_av,
            tc.tile_pool(name="ps_misc", bufs=1, space="PSUM") as ps_misc,
        ):
            # ---- resident tensors -------------------------------------
            wq_sb = singles.tile([128, ECH, 128], bf16, tag="wq")
            wk_sb = singles.tile([128, ECH, 128], bf16, tag="wk")
            wv_sb = singles.tile([128, ECH, HPC * 9], bf16, tag="wv")
            msk_sb = singles.tile([128, 2, 128], bf16, tag="msk")
            # startup: K path fully on the Sync DMA queue, Q path (incl. its
            # weight) on GpSimd, so both projection chains run in parallel;
            # wv/msk are deferred until after the q tiles are queued
            nc.gpsimd.dma_start(out=wq_sb, in_=wq.rearrange("p (c m) -> p c m", c=ECH))
            nc.sync.dma_start(out=wk_sb, in_=wk.rearrange("p (c m) -> p c m", c=ECH))

            def late_weights():
                nc.gpsimd.dma_start(
                    out=wv_sb, in_=wv.rearrange("p (c m) -> p c m", c=ECH)
                )
                nc.gpsimd.dma_start(
                    out=msk_sb, in_=msk.rearrange("p (g n) -> p g n", g=2)
                )

            # per-chunk projected tensors (separate tiles so the tile
            # dependency tracker never serializes chunk c's reads against
            # chunk c+2's writes)
            KT_t = [
                singles.tile([128, 512], bf16, tag=f"KT{c}", name=f"KT{c}")
                for c in range(NQC)
            ]
            QT_t = [
                singles.tile([128, 512], bf16, tag=f"QT{c}", name=f"QT{c}")
                for c in range(NQC)
            ]
            V_t = [
                singles.tile([128, 4, HPC, 9], bf16, tag=f"V{c}", name=f"V{c}")
                for c in range(NQC)
            ]

            ones9 = singles.tile([128, 9], bf16, tag="ones9")
            nc.vector.memset(ones9, 1.0)
            for c in range(NQC):
                nc.vector.memset(V_t[c][:, :, :, 0:1], 1.0)

            dmaq = [nc.sync, nc.gpsimd]

            def proj_pieces(c, lo_first=False):
                """Q/K/V projections for chunk c as dicts of emission
                closures (<=0.7us of tensor work each) so pieces fit a
                slot's tensor headroom without starving the exp pipeline.
                With lo_first, the q/k pieces emit only the [0:64] cast
                (heads 0-1); the [64:128] casts are exposed as "q_hi" /
                "k_hi" closures so the first score group can start before
                the second half is cast."""
                cs = slice(c * 512, (c + 1) * 512)
                st = {}

                def dma_in(src, key, n):
                    tiles = []
                    for e in range(ECH):
                        if c == 0 and key in ("k", "q"):
                            q = dmaq[0 if key == "k" else 1]
                        else:
                            q = dmaq[(e + n) % 2]
                        t = loads.tile([128, 512], bf16, tag="ld",
                                       name=f"{key}{c}_{e}")
                        r0 = (c * ECH + e) * 128
                        q.dma_start(out=t, in_=src[r0:r0 + 128, :])
                        tiles.append(t)
                    st[key] = tiles

                def cast_half(key, dst, g):
                    nc.vector.tensor_copy(
                        dst[64 * g:64 * g + 64, :],
                        st[key + "ps"][64 * g:64 * g + 64, :],
                    )

                st["cast"] = cast_half

                def qk_mm(key, w_sb, dst, e, casts=(0, 1)):
                    if e == 0:
                        if c == 0 and key in ("q", "k"):
                            # startup: borrow score-pool buffers (idle
                            # until the first scores) so the K and Q
                            # projection chains run in parallel despite
                            # ps_misc having a single buffer
                            qt0 = ps_sc.tile([128, 2, 512], f32, tag="sc",
                                             name=f"{key}ps0")
                            st[key + "ps"] = qt0[:, 0, :]
                        else:
                            st[key + "ps"] = ps_misc.tile(
                                [128, 512], f32, tag="ps", name=f"{key}ps{c}"
                            )
                    if c == 0 and key in ("q", "k"):
                        # startup: nothing else is in flight to co-stream
                        # with, so split into two column-tile halves that
                        # pair with each other (halves the wall time)
                        for g in range(2):
                            nc.tensor.matmul(
                                st[key + "ps"][64 * g:64 * g + 64, :],
                                w_sb[:, e, 64 * g:64 * g + 64],
                                st[key][e][:, :],
                                start=(e == 0), stop=(e == ECH - 1),
                                tile_position=(0, 64 * g),
                            )
                    else:
                        # steady state: solo full-width matmul — splitting
                        # is wall-neutral for the pair itself but blocks
                        # BOTH PE stream lanes; a solo leaves the second
                        # lane free for score/AV pairs to ride
                        nc.tensor.matmul(
                            st[key + "ps"], w_sb[:, e, :], st[key][e][:, :],
                            start=(e == 0), stop=(e == ECH - 1),
                        )
                    if e == ECH - 1:
                        # split the cast so the first score batch (heads
                        # 0-1, partitions < 64) gates on the first half
                        for g in casts:
                            cast_half(key, dst, g)

                def v_mm(tb, half):
                    if half == 0:
                        st[f"vps{tb}"] = ps_misc.tile(
                            [128, HPC * 9], f32, tag="ps", name=f"vps{c}_{tb}"
                        )
                    for e in (0, 1) if half == 0 else (2, 3):
                        nc.tensor.matmul(
                            st[f"vps{tb}"],
                            st["v"][e][:, tb * 128:(tb + 1) * 128],
                            wv_sb[:, e, :],
                            start=(e == 0), stop=(e == ECH - 1),
                        )
                    if half == 1:
                        dst = V_t[c][:, tb, :, 1:9]
                        src = st[f"vps{tb}"].rearrange(
                            "p (h n) -> p h n", n=9
                        )[:, :, 1:9]
                        nc.vector.tensor_copy(dst, src)

                qkcasts = (0,) if lo_first else (0, 1)
                return {
                    "q": [
                        lambda e=e: (
                            dma_in(qT, "q", 0) if e == 0 else None,
                            qk_mm("q", wq_sb, QT_t[c], e, qkcasts),
                        )
                        for e in range(ECH)
                    ],
                    "k": [
                        lambda e=e: (
                            dma_in(kT, "k", 1) if e == 0 else None,
                            qk_mm("k", wk_sb, KT_t[c], e, qkcasts),
                        )
                        for e in range(ECH)
                    ],
                    "q_hi": lambda: cast_half("q", QT_t[c], 1),
                    "k_hi": lambda: cast_half("k", KT_t[c], 1),
                    "v": [
                        lambda tb=tb, half=half: (
                            dma_in(vT, "v", 0)
                            if (tb == 0 and half == 0) else None,
                            v_mm(tb, half),
                        )
                        for tb in range(4)
                        for half in range(2)
                    ],
                }

            # ---- attention, software-pipelined ------------------------
            def emit_sa(c, tb, gs=(0, 1), ats=None):
                """Scores + mask + exp for (chunk c, t-block tb), head
                groups `gs` (pass ats back in to finish a partial block)."""
                d = 128 * tb - 512 * c  # diagonal offset within chunk
                vstart = max(d, 0)
                if ats is None:
                    ats = [None, None]
                scg = {}
                # emit ALL score matmuls before the activations: the four
                # row-disjoint tiles can co-stream wider than pairwise when
                # they queue together
                for g in gs:
                    scg[g] = ps_sc.tile([128, 2, 512], f32, tag="sc",
                                        name=f"sc{c}_{tb}_{g}")
                    ats[g] = abuf.tile([128, 2, 512], bf16, tag="a",
                                       name=f"a{c}_{tb}_{g}")
                    for j in range(2):
                        h = 2 * g + j
                        nc.tensor.matmul(
                            scg[g][:, j, vstart:512],
                            KT_t[tb // 4][32 * h:32 * h + 8,
                                          (tb % 4) * 128:(tb % 4 + 1) * 128],
                            QT_t[c][32 * h:32 * h + 8, vstart:512],
                            start=True, stop=True,
                            tile_position=(32 * h, 0),
                        )
                for g in gs:
                    nc.scalar.activation(
                        ats[g][:, :, vstart:512], scg[g][:, :, vstart:512],
                        Exp, scale=SCALE,
                    )
                    if d >= 0:
                        # zero the upper triangle AFTER exp (bf16 SBUF mul
                        # runs in the DVE 2x mode and keeps the scores->exp
                        # chain free of DVE hops)
                        nc.vector.tensor_mul(
                            ats[g][:, :, d:d + 128],
                            ats[g][:, :, d:d + 128],
                            msk_sb,
                        )
                return ats, vstart

            # prefix: chunk 0's K/Q projections inline, first scores/exp,
            # then chunk 0's first V block and chunk 1's Q/K — all during
            # the act-idle startup window
            sa_pre = {}  # (c, tb) -> (ats, vstart) emitted ahead of its chunk
            p0 = proj_pieces(0, lo_first=True)
            pp = {c: proj_pieces(c) for c in range(1, NQC)}
            for f in p0["k"]:
                f()
            for f in p0["q"]:
                f()
            late_weights()
            # preload the exp activation table (1.3us) on the scalar queue
            # AFTER its startup DMA issues, still ahead of the first scores
            warm = abuf.tile([128, 2], bf16, tag="warm")
            nc.scalar.activation(warm, ones9[:, 0:2], Exp, scale=1.0)
            # first score group fires on the lo casts alone; the hi casts
            # run on DVE while the act engine chews on group 0
            ats00, vs00 = emit_sa(0, 0, gs=(0,))
            p0["k_hi"]()
            p0["q_hi"]()
            emit_sa(0, 0, gs=(1,), ats=ats00)
            sa_pre[(0, 0)] = (ats00, vs00)
            p0["v"][0]()
            p0["v"][1]()
            for f in pp[1]["q"]:
                f()
            for f in pp[1]["k"]:
                f()

            work = []  # FIFO of deferred emission closures
            avs = {}
            for c in range(NQC):
                # no memset: every AV chain starts with start=True, and the
                # host only reads rows 32h..32h+9, which are always written
                av = avs[c] = ps_av.tile([128, 512], f32, tag="av",
                                         name=f"av{c}")
                ntb = 4 * (c + 1)

                if c == 0:
                    work.extend(p0["v"][2:8])  # V1-3(0)
                else:
                    # q(c+1) BEFORE v(c): q's final cast gates the next
                    # chunk's prefetched scores at the boundary, while
                    # v(c)'s last pieces are only read by this chunk's
                    # final AV batch
                    if c >= 2:
                        work.extend(pp[c]["k"])
                    if c + 1 < NQC:
                        work.extend(pp[c + 1]["q"])
                    else:
                        work.extend([None, None, None])
                    work.extend(pp[c]["v"])

                pend = None  # (tb, ats, vstart) waiting for its AV matmuls
                for tb in range(ntb):
                    if (c, tb) in sa_pre:
                        ats, vstart = sa_pre.pop((c, tb))
                    else:
                        ats, vstart = emit_sa(c, tb)
                    # prefetch the next chunk's first TWO score/exp groups
                    # from inside the LAST slot — ahead of AV(ntb-2), the
                    # final AV batch and the pieces — so the act pipeline
                    # never drains at the boundary
                    if tb == ntb - 1 and c + 1 < NQC:
                        sa_pre[(c + 1, 0)] = emit_sa(c + 1, 0)
                        sa_pre[(c + 1, 1)] = emit_sa(c + 1, 1)
                    # AV for the PREVIOUS tb — keeps exp ahead of the PE
                    if pend is not None:
                        ptb, pats, pvs = pend
                        for h in range(HPC):
                            g, j = divmod(h, 2)
                            nc.tensor.matmul(
                                av[32 * h:32 * h + 9, pvs:512],
                                V_t[ptb // 4][:, ptb % 4, h, :],
                                pats[g][:, j, pvs:512],
                                start=(ptb == 0), stop=False,
                                tile_position=(0, 32 * h),
                            )
                    pend = (tb, ats, vstart)
                    # drain the FIFO evenly across this chunk's slots,
                    # holding back the first 3 slots (c>0) so the boundary
                    # scores are never stuck behind pieces
                    hold = 0 if c == 0 else 3
                    if tb >= hold:
                        npop = -(-len(work) // (ntb - tb))  # ceil
                        for _ in range(npop):
                            piece = work.pop(0)
                            if piece is not None:
                                piece()
                av_sb = outs.tile([128, 512], f32, tag="avsb",
                                  name=f"avsb{c}")
                if c == NQC - 1:
                    # tail: columns [0, pvs) are final once AV(ntb-2) is in
                    # (the last block only touches [pvs, 512)) — ship them
                    # now so the drain doesn't wait on the big transfer
                    lo = pend[2]  # = vstart of the final t-block
                    nc.vector.tensor_copy(av_sb[:, 0:lo], av[:, 0:lo])
                    nc.sync.dma_start(
                        out=out[c * 128:(c + 1) * 128, 0:lo],
                        in_=av_sb[:, 0:lo]
                    )
                else:
                    lo = 0
                ptb, pats, pvs = pend
                for h in range(HPC):
                    g, j = divmod(h, 2)
                    nc.tensor.matmul(
                        av[32 * h:32 * h + 9, pvs:512],
                        V_t[ptb // 4][:, ptb % 4, h, :],
                        pats[g][:, j, pvs:512],
                        start=(ptb == 0), stop=True,
                        tile_position=(0, 32 * h),
                    )
                # ship the raw accumulator (numerators + denominators):
                # PSUM -> SBUF copy on DVE, then DMA out.  Emitted here
                # (before the next chunk's av memset in DVE program order)
                # so the bufs=2 rotation can never deadlock.
                nc.vector.tensor_copy(av_sb[:, lo:512], av[:, lo:512])
                # last chunk's store goes on the Sync queue: the exit
                # drain sequence ends on gpsimd, which then doesn't wait
                # on the final output transfer
                (nc.sync if c == NQC - 1 else dmaq[c % 2]).dma_start(
                    out=out[c * 128:(c + 1) * 128, lo:512],
                    in_=av_sb[:, lo:512]
                )
    _split_multi_waits(nc)
    return nc


def _prep_inputs(query, key, value, Wq, Wk, Wv):
    """Build the 8 per-core input maps (host-side sharding/layout)."""
    import ml_dtypes

    bf16 = np.dtype(ml_dtypes.bfloat16)

    def packT(x):  # [S, E] -> tile-packed [(chunk e p), 512] bf16
        xt = np.ascontiguousarray(x.T)  # [E, S]
        return (
            xt.reshape(ECH, 128, NQC, 512).transpose(2, 0, 1, 3)
            .astype(bf16).reshape(ECH * NQC * 128, 512)
        )

    qTs = [packT(query[b]) for b in range(B)]
    kTs = [packT(key[b]) for b in range(B)]
    vTs = [packT(value[b]) for b in range(B)]

    mask = np.where(
        np.arange(128)[:, None] <= np.arange(128)[None, :], 1.0, 0.0
    ).astype(np.float32)
    msk2 = np.ascontiguousarray(np.tile(mask, (1, 2))).astype(bf16)

    in_maps = []
    for core in range(NCORES):
        b, hh = divmod(core, 2)
        wq_p = np.zeros((E, 128), np.float32)
        wk_p = np.zeros((E, 128), np.float32)
        wv_p = np.zeros((E, HPC * 9), np.float32)
        for h in range(HPC):
            g = 4 * hh + h
            wq_p[:, 32 * h:32 * h + 8] = Wq[g]
            wk_p[:, 32 * h:32 * h + 8] = Wk[g]
            wv_p[:, 9 * h + 1:9 * h + 9] = Wv[g]
        def pack(w):  # [E, M] -> [128, ECH*M] partition-major
            m = w.shape[1]
            return np.ascontiguousarray(
                w.reshape(ECH, 128, m).transpose(1, 0, 2).reshape(128, ECH * m)
            )

        in_maps.append(
            {
                "qT": qTs[b], "kT": kTs[b], "vT": vTs[b],
                "wq": pack(wq_p).astype(bf16), "wk": pack(wk_p).astype(bf16),
                "wv": pack(wv_p).astype(bf16),
                "msk": msk2,
            }
        )
    return in_maps


def _reference_numpy(query, key, value, padding_mask, decoder_mask,
                     Wq, Wk, Wv, Wo, bo):
    """Fallback (non-default masks): plain numpy replica of the reference."""
    q = np.einsum("bse,hed->bhsd", query, Wq)
    k = np.einsum("bse,hed->bhsd", key, Wk)
    v = np.einsum("bse,hed->bhsd", value, Wv)
    s = np.einsum("bhsd,bhtd->bhst", q, k)
    if decoder_mask:
        tril = np.tril(s)
        s = np.where(tril == 0.0, -np.inf, s)
    s = np.where(padding_mask[:, None, :, :], s, -np.inf)
    s = s / np.sqrt(np.float32(DK_H))
    m = np.max(s, axis=-1, keepdims=True)
    e = np.exp(s - m)
    a = e / np.sum(e, axis=-1, keepdims=True)
    o = np.einsum("bhst,bhtd->bhsd", a, v)
    o = o.transpose(0, 2, 1, 3).reshape(o.shape[0], o.shape[2], H * DV_H)
    return (o @ Wo + bo).astype(np.float32)


def kernel(query, key, value, padding_mask, decoder_mask, Wq, Wk, Wv, Wo, bo,
           **run_kwargs):
    query = np.asarray(query, np.float32)
    key = np.asarray(key, np.float32)
    value = np.asarray(value, np.float32)
    Wq = np.asarray(Wq, np.float32)
    Wk = np.asarray(Wk, np.float32)
    Wv = np.asarray(Wv, np.float32)
    Wo = np.asarray(Wo, np.float32)
    bo = np.asarray(bo, np.float32)
    pm = np.asarray(padding_mask)
    dm = int(np.asarray(decoder_mask))

    if not bool(pm.all()) or not dm:
        return _reference_numpy(
            query, key, value, pm.astype(bool), dm, Wq, Wk, Wv, Wo, bo
        )

    from concourse.bass_utils import run_bass_kernel_spmd

    if "nc" not in _cache:
        _cache["nc"] = _build()
    nc = _cache["nc"]

    in_maps = _prep_inputs(query, key, value, Wq, Wk, Wv)
    res = run_bass_kernel_spmd(nc, in_maps, list(range(NCORES)), **run_kwargs)

    # host finish: divide numerators by denominators, concatenate heads,
    # apply the output projection + bias (all f32)
    outp = np.empty((B, S, E), np.float32)
    ov = np.empty((S, H * DV_H), np.float32)
    for b in range(B):
        for hh in range(2):
            # chunk-major [NQC*128, 512] -> [128, S]
            r = res.results[2 * b + hh]["out"]
            r = r.reshape(NQC, 128, 512).transpose(1, 0, 2).reshape(128, S)
            for h in range(HPC):
                g = 4 * hh + h
                den = r[32 * h]
                num = r[32 * h + 1:32 * h + 9]
                ov[:, 8 * g:8 * g + 8] = (num / den).T
        outp[b] = ov @ Wo + bo
    if run_kwargs:
        kernel.last_result = res
    return outp


# revision 71
# speedup vs baseline: 1.1270x; 1.0051x over previous
"""Bass/Trainium2 kernel for nn_MultiHeadAttention (B=4, S=2048, E=512, H=8, dk=dv=8).

Sharding: 8 cores = (batch b, head-half hh).  Core 2b+hh computes causal
attention for batch b over heads [4hh, 4hh+4) for all 2048 queries and
returns the UNNORMALIZED attention accumulator per chunk: rows 32h hold
the softmax denominator (ones column in V), rows 32h+1..32h+9 the
numerator (exp(s) @ V_h).  The host divides, concatenates heads, and
applies the output projection + bias in f32 (cheap: 0.5 GFLOP total).

Device layout notes (v4 — host-finish, deep score buffering, packed DMA):
  - Host feeds query/key/value as host-packed CONTIGUOUS [128, 512] bf16
    tiles, (chunk, e)-major, so every input DMA is a single 128KB burst
    (the [E, S] view's 1KB-of-4KB strided reads throttled startup).
  - Projection weights are host-packed bf16 "spread" layouts: Q/K outputs
    land at partitions {32h+d}; V outputs at columns {9h+d} with a ones
    column per head at 9h+0 accumulating the softmax denominator.
  - The PE power throttle caps each matmul stream at ~0.78 cols/ns, but
    two tile-disjoint matmuls co-stream at full rate each (measured; >2
    rarely co-streams, and matmul PSUM destinations must be 2KB-bank-
    aligned, which forbids packing 4 quad-issued heads into fewer banks).
    Scores pair via row-tiling (tile_position=(32h, 0), K=8); A@V pairs
    via col-tiling (tile_position=(0, 32h), M=9) and sits at the PE's
    ~205 G elem/s element cap.  Q/K projections run as full-width solos
    in steady state (splitting them only burns the second stream lane);
    chunk 0's run as co-streaming column-tile halves since nothing else
    is in flight at startup.
  - Scores are computed transposed ([t, q]); exp outputs bf16 tiles that
    feed the A@V matmul as the moving operand (V stationary).  The causal
    mask is applied AFTER exp as a bf16 0/1 multiply (keeps the
    scores->exp chain free of DVE hops).
  - PSUM: score tiles [128, 2, 512] f32 x3 bufs (6 banks) so a score
    pair is released three activations back; av accumulator x1 (no
    memset — AV chains start with start=True and the host ignores
    unwritten rows); projection psum x1 (chunk-0's K/Q borrow idle
    score-pool buffers to keep their chains parallel at startup).
  - Emission is software-pipelined: scores(tb+1) are emitted before
    AV(tb); projections are diced into <=0.7us closures drained via a
    work FIFO ordered [k(c), q(c+1), v(c)] (q's final cast gates the
    next chunk's prefetched scores; v's last pieces are only read by the
    final AV batch).  Each chunk ends with a DVE copy of the raw AV
    accumulator PSUM->SBUF and a contiguous [128, 512] f32 store; the
    last chunk ships columns [0, 384) right after AV(14) so the drain
    only waits on a 64KB tail.
"""

import math

import numpy as np

B, S, E, H = 4, 2048, 512, 8
DK_H = DV_H = 8
NCORES = 8
HPC = H // 2  # heads per core = 4
SCALE = 1.0 / math.sqrt(DK_H)
NQC = S // 512  # q chunks of 512
NTB = S // 128  # t blocks of 128
ECH = E // 128  # e chunks of 128

_cache: dict = {}


def _apply_tile_patch():
    """walrus in this image allows only one sync-wait per Drain; split the
    TileContext tail drain's waits across a chain of drains."""
    import concourse.mybir as mybir
    from concourse import tile
    from concourse.vector_clock import ScopedClock

    if getattr(tile.TileContext._drain_and_barrier, "_split_patch", False):
        return

    def _drain_and_barrier_split(self, tick_clock, wait_clock):
        drain_inst = self.nc.sync.drain()
        wait_clock.add_sem_waits(
            drain_inst.ins, ScopedClock({None: tick_clock.global_clock})
        )
        si = drain_inst.ins.sync_info
        if si is not None and si.on_wait and len(si.on_wait) > 1:
            waits = list(si.on_wait)
            si.on_wait = waits[:1]
            for entry in waits[1:]:
                extra = self.nc.sync.drain()
                extra.ins.sync_info = mybir.SyncInfo(on_wait=[entry], on_update=[])
        self.nc.all_engine_barrier()
        assert self.sems is not None
        popped = self.nc._tile_sem_poison_stack.pop()
        assert popped is self._sem_poison
        self.nc.clear_and_free_semaphores(list(self.sems.allocated().values()))
        self.nc.all_engine_barrier()

    _drain_and_barrier_split._split_patch = True
    tile.TileContext._drain_and_barrier = _drain_and_barrier_split


def _split_multi_waits(nc):
    """walrus in this image allows only one sync-wait per instruction;
    move excess waits onto single-wait NOPs inserted just before."""
    import concourse.mybir as mybir

    for blk in nc.m.functions[0].blocks:
        out = []
        for inst in blk.instructions:
            si = getattr(inst, "sync_info", None)
            if si is not None and si.on_wait and len(si.on_wait) > 1:
                waits = list(si.on_wait)
                for i, entry in enumerate(waits[:-1]):
                    out.append(
                        mybir.InstNoOp(
                            name=f"{inst.name}_w{i}",
                            engine=inst.engine,
                            ins=[],
                            outs=[],
                            bass_nofuse=True,
                            sync_info=mybir.SyncInfo(
                                on_wait=[entry], on_update=[]
                            ),
                        )
                    )
                si.on_wait = waits[-1:]
            out.append(inst)
        blk.instructions = out


def _build():
    import concourse.bass as bassmod
    import concourse.mybir as mybir
    from concourse import tile

    _apply_tile_patch()
    f32 = mybir.dt.float32
    bf16 = mybir.dt.bfloat16
    Exp = mybir.ActivationFunctionType.Exp

    nc = bassmod.Bass()
    # q/k/v host-packed into contiguous [128, 512] tiles, (e, chunk)-major,
    # so every input DMA is one contiguous 128KB burst instead of a
    # 1KB-of-4KB strided read
    qT = nc.declare_dram_parameter("qT", [ECH * NQC * 128, 512], bf16,
                                   isOutput=False)
    kT = nc.declare_dram_parameter("kT", [ECH * NQC * 128, 512], bf16,
                                   isOutput=False)
    vT = nc.declare_dram_parameter("vT", [ECH * NQC * 128, 512], bf16,
                                   isOutput=False)
    # weights host-packed partition-major so their DMAs are contiguous
    wq = nc.declare_dram_parameter("wq", [128, ECH * 128], bf16, isOutput=False)
    wk = nc.declare_dram_parameter("wk", [128, ECH * 128], bf16, isOutput=False)
    wv = nc.declare_dram_parameter("wv", [128, ECH * HPC * 9], bf16,
                                   isOutput=False)
    msk = nc.declare_dram_parameter("msk", [128, 2 * 128], bf16, isOutput=False)
    # output chunk-major: each chunk's [128, 512] store is contiguous
    out = nc.declare_dram_parameter("out", [NQC * 128, 512], f32,
                                    isOutput=True)

    with tile.TileContext(nc) as tc:
        with (
            tc.tile_pool(name="singles", bufs=1) as singles,
            tc.tile_pool(name="loads", bufs=16) as loads,
            tc.tile_pool(name="abuf", bufs=14) as abuf,
            tc.tile_pool(name="outs", bufs=4) as outs,
            tc.tile_pool(name="ps_sc", bufs=3, space="PSUM") as ps_sc,
            tc.tile_pool(name="ps_av", bufs=1, space="PSUM") as ps_av,
            tc.tile_pool(name="ps_misc", bufs=1, space="PSUM") as ps_misc,
        ):
            # ---- resident tensors -------------------------------------
            wq_sb = singles.tile([128, ECH, 128], bf16, tag="wq")
            wk_sb = singles.tile([128, ECH, 128], bf16, tag="wk")
            wv_sb = singles.tile([128, ECH, HPC * 9], bf16, tag="wv")
            msk_sb = singles.tile([128, 2, 128], bf16, tag="msk")
            # startup: K path fully on the Sync DMA queue, Q path (incl. its
            # weight) on GpSimd, so both projection chains run in parallel;
            # wv/msk are deferred until after the q tiles are queued
            nc.gpsimd.dma_start(out=wq_sb, in_=wq.rearrange("p (c m) -> p c m", c=ECH))
            nc.sync.dma_start(out=wk_sb, in_=wk.rearrange("p (c m) -> p c m", c=ECH))

            def late_weights():
                nc.gpsimd.dma_start(
                    out=wv_sb, in_=wv.rearrange("p (c m) -> p c m", c=ECH)
                )
                nc.gpsimd.dma_start(
                    out=msk_sb, in_=msk.rearrange("p (g n) -> p g n", g=2)
                )

            # per-chunk projected tensors (separate tiles so the tile
            # dependency tracker never serializes chunk c's reads against
            # chunk c+2's writes)
            KT_t = [
                singles.tile([128, 512], bf16, tag=f"KT{c}", name=f"KT{c}")
                for c in range(NQC)
            ]
            QT_t = [
                singles.tile([128, 512], bf16, tag=f"QT{c}", name=f"QT{c}")
                for c in range(NQC)
            ]
            V_t = [
                singles.tile([128, 4, HPC, 9], bf16, tag=f"V{c}", name=f"V{c}")
                for c in range(NQC)
            ]

            ones9 = singles.tile([128, 9], bf16, tag="ones9")
            nc.vector.memset(ones9, 1.0)
            for c in range(NQC):
                nc.vector.memset(V_t[c][:, :, :, 0:1], 1.0)

            dmaq = [nc.sync, nc.gpsimd]

            def proj_pieces(c, lo_first=False):
                """Q/K/V projections for chunk c as dicts of emission
                closures (<=0.7us of tensor work each) so pieces fit a
                slot's tensor headroom without starving the exp pipeline.
                With lo_first, the q/k pieces emit only the [0:64] cast
                (heads 0-1); the [64:128] casts are exposed as "q_hi" /
                "k_hi" closures so the first score group can start before
                the second half is cast."""
                cs = slice(c * 512, (c + 1) * 512)
                st = {}

                def dma_in(src, key, n):
                    tiles = []
                    for e in range(ECH):
                        if c == 0 and key in ("k", "q"):
                            q = dmaq[0 if key == "k" else 1]
                        else:
                            q = dmaq[(e + n) % 2]
                        t = loads.tile([128, 512], bf16, tag="ld",
                                       name=f"{key}{c}_{e}")
                        r0 = (c * ECH + e) * 128
                        q.dma_start(out=t, in_=src[r0:r0 + 128, :])
                        tiles.append(t)
                    st[key] = tiles

                def cast_half(key, dst, g):
                    nc.vector.tensor_copy(
                        dst[64 * g:64 * g + 64, :],
                        st[key + "ps"][64 * g:64 * g + 64, :],
                    )

                st["cast"] = cast_half

                def qk_mm(key, w_sb, dst, e, casts=(0, 1)):
                    if e == 0:
                        if c == 0 and key in ("q", "k"):
                            # startup: borrow score-pool buffers (idle
                            # until the first scores) so the K and Q
                            # projection chains run in parallel despite
                            # ps_misc having a single buffer
                            qt0 = ps_sc.tile([128, 2, 512], f32, tag="sc",
                                             name=f"{key}ps0")
                            st[key + "ps"] = qt0[:, 0, :]
                        else:
                            st[key + "ps"] = ps_misc.tile(
                                [128, 512], f32, tag="ps", name=f"{key}ps{c}"
                            )
                    if c == 0 and key in ("q", "k"):
                        # startup: nothing else is in flight to co-stream
                        # with, so split into two column-tile halves that
                        # pair with each other (halves the wall time)
                        for g in range(2):
                            nc.tensor.matmul(
                                st[key + "ps"][64 * g:64 * g + 64, :],
                                w_sb[:, e, 64 * g:64 * g + 64],
                                st[key][e][:, :],
                                start=(e == 0), stop=(e == ECH - 1),
                                tile_position=(0, 64 * g),
                            )
                    else:
                        # steady state: solo full-width matmul — splitting
                        # is wall-neutral for the pair itself but blocks
                        # BOTH PE stream lanes; a solo leaves the second
                        # lane free for score/AV pairs to ride
                        nc.tensor.matmul(
                            st[key + "ps"], w_sb[:, e, :], st[key][e][:, :],
                            start=(e == 0), stop=(e == ECH - 1),
                        )
                    if e == ECH - 1:
                        # split the cast so the first score batch (heads
                        # 0-1, partitions < 64) gates on the first half
                        for g in casts:
                            cast_half(key, dst, g)

                def v_mm(tb, half):
                    if half == 0:
                        st[f"vps{tb}"] = ps_misc.tile(
                            [128, HPC * 9], f32, tag="ps", name=f"vps{c}_{tb}"
                        )
                    for e in (0, 1) if half == 0 else (2, 3):
                        nc.tensor.matmul(
                            st[f"vps{tb}"],
                            st["v"][e][:, tb * 128:(tb + 1) * 128],
                            wv_sb[:, e, :],
                            start=(e == 0), stop=(e == ECH - 1),
                        )
                    if half == 1:
                        dst = V_t[c][:, tb, :, 1:9]
                        src = st[f"vps{tb}"].rearrange(
                            "p (h n) -> p h n", n=9
                        )[:, :, 1:9]
                        nc.vector.tensor_copy(dst, src)

                qkcasts = (0,) if lo_first else (0, 1)
                return {
                    "q": [
                        lambda e=e: (
                            dma_in(qT, "q", 0) if e == 0 else None,
                            qk_mm("q", wq_sb, QT_t[c], e, qkcasts),
                        )
                        for e in range(ECH)
                    ],
                    "k": [
                        lambda e=e: (
                            dma_in(kT, "k", 1) if e == 0 else None,
                            qk_mm("k", wk_sb, KT_t[c], e, qkcasts),
                        )
                        for e in range(ECH)
                    ],
                    "q_hi": lambda: cast_half("q", QT_t[c], 1),
                    "k_hi": lambda: cast_half("k", KT_t[c], 1),
                    "v": [
                        lambda tb=tb, half=half: (
                            dma_in(vT, "v", 0)
                            if (tb == 0 and half == 0) else None,
                            v_mm(tb, half),
                        )
                        for tb in range(4)
                        for half in range(2)
                    ],
                }

            # ---- attention, software-pipelined ------------------------
            def emit_sa(c, tb, gs=(0, 1), ats=None):
                """Scores + mask + exp for (chunk c, t-block tb), head
                groups `gs` (pass ats back in to finish a partial block)."""
                d = 128 * tb - 512 * c  # diagonal offset within chunk
                vstart = max(d, 0)
                if ats is None:
                    ats = [None, None]
                scg = {}
                # emit ALL score matmuls before the activations: the four
                # row-disjoint tiles can co-stream wider than pairwise when
                # they queue together
                for g in gs:
                    scg[g] = ps_sc.tile([128, 2, 512], f32, tag="sc",
                                        name=f"sc{c}_{tb}_{g}")
                    ats[g] = abuf.tile([128, 2, 512], bf16, tag="a",
                                       name=f"a{c}_{tb}_{g}")
                    for j in range(2):
                        h = 2 * g + j
                        nc.tensor.matmul(
                            scg[g][:, j, vstart:512],
                            KT_t[tb // 4][32 * h:32 * h + 8,
                                          (tb % 4) * 128:(tb % 4 + 1) * 128],
                            QT_t[c][32 * h:32 * h + 8, vstart:512],
                            start=True, stop=True,
                            tile_position=(32 * h, 0),
                        )
                for g in gs:
                    nc.scalar.activation(
                        ats[g][:, :, vstart:512], scg[g][:, :, vstart:512],
                        Exp, scale=SCALE,
                    )
                    if d >= 0:
                        # zero the upper triangle AFTER exp (bf16 SBUF mul
                        # runs in the DVE 2x mode and keeps the scores->exp
                        # chain free of DVE hops)
                        nc.vector.tensor_mul(
                            ats[g][:, :, d:d + 128],
                            ats[g][:, :, d:d + 128],
                            msk_sb,
                        )
                return ats, vstart

            # prefix: chunk 0's K/Q projections inline, first scores/exp,
            # then chunk 0's first V block and chunk 1's Q/K — all during
            # the act-idle startup window
            sa_pre = {}  # (c, tb) -> (ats, vstart) emitted ahead of its chunk
            p0 = proj_pieces(0, lo_first=True)
            pp = {c: proj_pieces(c) for c in range(1, NQC)}
            for f in p0["k"]:
                f()
            for f in p0["q"]:
                f()
            late_weights()
            # preload the exp activation table (1.3us) on the scalar queue
            # AFTER its startup DMA issues, still ahead of the first scores
            warm = abuf.tile([128, 2], bf16, tag="warm")
            nc.scalar.activation(warm, ones9[:, 0:2], Exp, scale=1.0)
            # first score group fires on the lo casts alone; the hi casts
            # run on DVE while the act engine chews on group 0
            ats00, vs00 = emit_sa(0, 0, gs=(0,))
            p0["k_hi"]()
            p0["q_hi"]()
            emit_sa(0, 0, gs=(1,), ats=ats00)
            sa_pre[(0, 0)] = (ats00, vs00)
            p0["v"][0]()
            p0["v"][1]()
            for f in pp[1]["q"]:
                f()
            for f in pp[1]["k"]:
                f()

            work = []  # FIFO of deferred emission closures
            avs = {}
            for c in range(NQC):
                # no memset: every AV chain starts with start=True, and the
                # host only reads rows 32h..32h+9, which are always written
                av = avs[c] = ps_av.tile([128, 512], f32, tag="av",
                                         name=f"av{c}")
                ntb = 4 * (c + 1)

                if c == 0:
                    work.extend(p0["v"][2:8])  # V1-3(0)
                else:
                    # q(c+1) BEFORE v(c): q's final cast gates the next
                    # chunk's prefetched scores at the boundary, while
                    # v(c)'s last pieces are only read by this chunk's
                    # final AV batch
                    if c >= 2:
                        work.extend(pp[c]["k"])
                    if c + 1 < NQC:
                        work.extend(pp[c + 1]["q"])
                    else:
                        work.extend([None, None, None])
                    work.extend(pp[c]["v"])

                pend = None  # (tb, ats, vstart) waiting for its AV matmuls
                for tb in range(ntb):
                    if (c, tb) in sa_pre:
                        ats, vstart = sa_pre.pop((c, tb))
                    else:
                        ats, vstart = emit_sa(c, tb)
                    # prefetch the next chunk's first TWO score/exp groups
                    # from inside the LAST slot — ahead of AV(ntb-2), the
                    # final AV batch and the pieces — so the act pipeline
                    # never drains at the boundary
                    if tb == ntb - 1 and c + 1 < NQC:
                        sa_pre[(c + 1, 0)] = emit_sa(c + 1, 0)
                        sa_pre[(c + 1, 1)] = emit_sa(c + 1, 1)
                    # AV for the PREVIOUS tb — keeps exp ahead of the PE
                    if pend is not None:
                        ptb, pats, pvs = pend
                        for h in range(HPC):
                            g, j = divmod(h, 2)
                            nc.tensor.matmul(
                                av[32 * h:32 * h + 9, pvs:512],
                                V_t[ptb // 4][:, ptb % 4, h, :],
                                pats[g][:, j, pvs:512],
                                start=(ptb == 0), stop=False,
                                tile_position=(0, 32 * h),
                            )
                    pend = (tb, ats, vstart)
                    # drain the FIFO evenly across this chunk's slots,
                    # holding back the first 3 slots (c>0) so the boundary
                    # scores are never stuck behind pieces
                    hold = 0 if c == 0 else 3
                    if tb >= hold:
                        npop = -(-len(work) // (ntb - tb))  # ceil
                        for _ in range(npop):
                            piece = work.pop(0)
                            if piece is not None:
                                piece()
                av_sb = outs.tile([128, 512], f32, tag="avsb",
                                  name=f"avsb{c}")
                if c == NQC - 1:
                    # tail: columns [0, pvs) are final once AV(ntb-2) is in
                    # (the last block only touches [pvs, 512)) — ship them
                    # now so the drain doesn't wait on the big transfer
                    lo = pend[2]  # = vstart of the final t-block
                    nc.vector.tensor_copy(av_sb[:, 0:lo], av[:, 0:lo])
                    nc.sync.dma_start(
                        out=out[c * 128:(c + 1) * 128, 0:lo],
                        in_=av_sb[:, 0:lo]
                    )
                else:
                    lo = 0
                ptb, pats, pvs = pend
                for h in range(HPC):
                    g, j = divmod(h, 2)
                    nc.tensor.matmul(
                        av[32 * h:32 * h + 9, pvs:512],
                        V_t[ptb // 4][:, ptb % 4, h, :],
                        pats[g][:, j, pvs:512],
                        start=(ptb == 0), stop=True,
                        tile_position=(0, 32 * h),
                    )
                # ship the raw accumulator (numerators + denominators):
                # PSUM -> SBUF copy on DVE, then DMA out.  Emitted here
                # (before the next chunk's av memset in DVE program order)
                # so the bufs=2 rotation can never deadlock.
                nc.vector.tensor_copy(av_sb[:, lo:512], av[:, lo:512])
                # last chunk's store goes on the Sync queue: the exit
                # drain sequence ends on gpsimd, which then doesn't wait
                # on the final output transfer
                (nc.sync if c == NQC - 1 else dmaq[c % 2]).dma_start(
                    out=out[c * 128:(c + 1) * 128, lo:512],
                    in_=av_sb[:, lo:512]
                )
    _split_multi_waits(nc)
    return nc


def _prep_inputs(query, key, value, Wq, Wk, Wv):
    """Build the 8 per-core input maps (host-side sharding/layout)."""
    import ml_dtypes

    bf16 = np.dtype(ml_dtypes.bfloat16)

    def packT(x):  # [S, E] -> tile-packed [(chunk e p), 512] bf16
        xt = np.ascontiguousarray(x.T)  # [E, S]
        return (
            xt.reshape(ECH, 128, NQC, 512).transpose(2, 0, 1, 3)
            .astype(bf16).reshape(ECH * NQC * 128, 512)
        )

    qTs = [packT(query[b]) for b in range(B)]
    kTs = [packT(key[b]) for b in range(B)]
    vTs = [packT(value[b]) for b in range(B)]

    mask = np.where(
        np.arange(128)[:, None] <= np.arange(128)[None, :], 1.0, 0.0
    ).astype(np.float32)
    msk2 = np.ascontiguousarray(np.tile(mask, (1, 2))).astype(bf16)

    in_maps = []
    for core in range(NCORES):
        b, hh = divmod(core, 2)
        wq_p = np.zeros((E, 128), np.float32)
        wk_p = np.zeros((E, 128), np.float32)
        wv_p = np.zeros((E, HPC * 9), np.float32)
        for h in range(HPC):
            g = 4 * hh + h
            wq_p[:, 32 * h:32 * h + 8] = Wq[g]
            wk_p[:, 32 * h:32 * h + 8] = Wk[g]
            wv_p[:, 9 * h + 1:9 * h + 9] = Wv[g]
        def pack(w):  # [E, M] -> [128, ECH*M] partition-major
            m = w.shape[1]
            return np.ascontiguousarray(
                w.reshape(ECH, 128, m).transpose(1, 0, 2).reshape(128, ECH * m)
            )

        in_maps.append(
            {
                "qT": qTs[b], "kT": kTs[b], "vT": vTs[b],
                "wq": pack(wq_p).astype(bf16), "wk": pack(wk_p).astype(bf16),
                "wv": pack(wv_p).astype(bf16),
                "msk": msk2,
            }
        )
    return in_maps


def _reference_numpy(query, key, value, padding_mask, decoder_mask,
                     Wq, Wk, Wv, Wo, bo):
    """Fallback (non-default masks): plain numpy replica of the reference."""
    q = np.einsum("bse,hed->bhsd", query, Wq)
    k = np.einsum("bse,hed->bhsd", key, Wk)
    v = np.einsum("bse,hed->bhsd", value, Wv)
    s = np.einsum("bhsd,bhtd->bhst", q, k)
    if decoder_mask:
        tril = np.tril(s)
        s = np.where(tril == 0.0, -np.inf, s)
    s = np.where(padding_mask[:, None, :, :], s, -np.inf)
    s = s / np.sqrt(np.float32(DK_H))
    m = np.max(s, axis=-1, keepdims=True)
    e = np.exp(s - m)
    a = e / np.sum(e, axis=-1, keepdims=True)
    o = np.einsum("bhst,bhtd->bhsd", a, v)
    o = o.transpose(0, 2, 1, 3).reshape(o.shape[0], o.shape[2], H * DV_H)
    return (o @ Wo + bo).astype(np.float32)


def kernel(query, key, value, padding_mask, decoder_mask, Wq, Wk, Wv, Wo, bo,
           **run_kwargs):
    query = np.asarray(query, np.float32)
    key = np.asarray(key, np.float32)
    value = np.asarray(value, np.float32)
    Wq = np.asarray(Wq, np.float32)
    Wk = np.asarray(Wk, np.float32)
    Wv = np.asarray(Wv, np.float32)
    Wo = np.asarray(Wo, np.float32)
    bo = np.asarray(bo, np.float32)
    pm = np.asarray(padding_mask)
    dm = int(np.asarray(decoder_mask))

    if not bool(pm.all()) or not dm:
        return _reference_numpy(
            query, key, value, pm.astype(bool), dm, Wq, Wk, Wv, Wo, bo
        )

    from concourse.bass_utils import run_bass_kernel_spmd

    if "nc" not in _cache:
        _cache["nc"] = _build()
    nc = _cache["nc"]

    in_maps = _prep_inputs(query, key, value, Wq, Wk, Wv)
    res = run_bass_kernel_spmd(nc, in_maps, list(range(NCORES)), **run_kwargs)

    # host finish: divide numerators by denominators, concatenate heads,
    # apply the output projection + bias (all f32)
    outp = np.empty((B, S, E), np.float32)
    ov = np.empty((S, H * DV_H), np.float32)
    for b in range(B):
        for hh in range(2):
            # chunk-major [NQC*128, 512] -> [128, S]
            r = res.results[2 * b + hh]["out"]
            r = r.reshape(NQC, 128, 512).transpose(1, 0, 2).reshape(128, S)
            for h in range(HPC):
                g = 4 * hh + h
                den = r[32 * h]
                num = r[32 * h + 1:32 * h + 9]
                ov[:, 8 * g:8 * g + 8] = (num / den).T
        outp[b] = ov @ Wo + bo
    if run_kwargs:
        kernel.last_result = res
    return outp
